# revision 61
# baseline (speedup 1.0000x reference)
"""Trainium2 Bass kernel for a dense transformer block (nn_Block_88338887344891).

Distribution over 8 NeuronCores (single SPMD NEFF, 2 collectives):
  - LN1 stats computed per-core on its own 512 tokens, AllGathered (48KB,
    ~16us, overlapped with the QKV matmuls on raw x).
  - LayerNorm is folded into the matmuls: QKV = inv * (W'^T x + s (-mu)) + b'
    where W' = diag(ln_w) W (host-folded), s = colsum(W'), so the heavy
    matmuls start before the stats arrive; ln_b is folded into b'.
  - QKV + causal attention head-sharded (2 heads/core over all 4096 tokens);
    scores/exp/AV trimmed to the causal region; V is produced directly in
    keys-major layout (x as the stationary operand) so no transposes.
  - attention output AllToAll per head in bf16 (1MB, ~41us)
  - output projection + residual + LN2 + full MLP token-sharded (512 tok/core)
  - gelu(tanh approx) via t*sigmoid(1.702 t) (1 ACT + 1 fused DVE op)
All matmul moving operands are bf16 (1 cycle/row); PSUM accumulates f32.
"""
import numpy as np
from contextlib import ExitStack

try:  # persistent XLA cache so repeat runs skip the NEFF compile
    import jax
    jax.config.update("jax_compilation_cache_dir", "/tmp/jax_neff_cache")
    jax.config.update("jax_persistent_cache_min_compile_time_secs", 1.0)
except Exception:
    pass

import ml_dtypes
import concourse.bass as bass
import concourse.bacc as bacc
import concourse.tile as tile
import concourse.mybir as mybir
from concourse import bass_utils

AF = mybir.ActivationFunctionType
ALU = mybir.AluOpType
F32 = mybir.dt.float32
F32R = mybir.dt.float32r
BF16 = mybir.dt.bfloat16
FP8 = mybir.dt.float8e4
NPBF16 = ml_dtypes.bfloat16
NPFP8 = mybir.dt.np(mybir.dt.float8e4)
DR = mybir.MatmulPerfMode.DoubleRow

NC_N = 8          # cores
B, T, D, H = 2, 2048, 1024, 16
HD = D // H       # 64
DFF = 4 * D       # 4096
EPS = 1e-5
BT = B * T               # 4096 tokens
TPC = BT // NC_N         # 512 tokens per core
HPC = H // NC_N          # 2 heads per core
PO = D // 128            # 8 D-tiles
M1 = DFF // 128          # 32 ff1 out tiles
NKT = BT // 128          # 32 key tiles globally (16 per batch)
RG = [list(range(NC_N))]
SIGC = 1.702             # gelu sigmoid-form constant

GELU_NATIVE = False   # kept for test.py compat; kernel is sim/hw identical

# Wo row order after the per-head AllToAll halves: feature index
# n = half*512 + po*128 + s2*64 + d maps to old row 64*(2*(2*po+s2)+half)+d
WO_PERM = np.array([64 * (2 * (2 * po + s2) + half) + d
                    for half in range(2) for po in range(4)
                    for s2 in range(2) for d in range(64)])

_CACHE = {}


def _build():
    nc = bacc.Bacc("TRN2", target_bir_lowering=False, debug=False,
                   num_devices=NC_N)

    # ---- per-core external inputs ----
    xb_in = nc.dram_tensor("xb", [D, BT], FP8, kind="ExternalInput")
    xs_in = nc.dram_tensor("xs", [D, TPC], BF16, kind="ExternalInput")
    xf_in = nc.dram_tensor("xf", [D, TPC], F32, kind="ExternalInput")
    wqkv_in = nc.dram_tensor("wqkv", [D, 768], FP8, kind="ExternalInput")
    sqkv_in = nc.dram_tensor("sqkv", [1, 256], F32R, kind="ExternalInput")
    svb_in = nc.dram_tensor("svb", [2, 128], BF16, kind="ExternalInput")
    bqkr_in = nc.dram_tensor("bqkr", [1, 256], F32R, kind="ExternalInput")
    wo_in = nc.dram_tensor("wo", [D, 2 * D], FP8, kind="ExternalInput")
    bo_in = nc.dram_tensor("bo", [1, D], BF16, kind="ExternalInput")
    wf1_in = nc.dram_tensor("wf1", [D, 2 * DFF], FP8, kind="ExternalInput")
    bf1_in = nc.dram_tensor("bf1", [128, M1], F32, kind="ExternalInput")
    bf1s_in = nc.dram_tensor("bf1s", [128, M1], F32, kind="ExternalInput")
    wf2_in = nc.dram_tensor("wf2", [DFF, 2 * D], FP8, kind="ExternalInput")
    bf2_in = nc.dram_tensor("bf2", [1, D], BF16, kind="ExternalInput")
    out_t = nc.dram_tensor("outt", [D, TPC], F32, kind="ExternalOutput")

    with tile.TileContext(nc, pool_alloc_mode="queue") as tc, \
            ExitStack() as ctx:
        perm = ctx.enter_context(tc.tile_pool(name="perm", bufs=1))
        big = ctx.enter_context(tc.tile_pool(name="big", bufs=1))
        rows = ctx.enter_context(tc.tile_pool(name="rows", bufs=1))
        dram = ctx.enter_context(tc.tile_pool(name="dram", bufs=1, space="DRAM"))

        # ---- constants ----
        ones_col_b = perm.tile([128, 1], BF16)
        nc.vector.memset(ones_col_b[:], 1.0)
        ones_col_f = perm.tile([128, 1], F32)
        nc.vector.memset(ones_col_f[:], 1.0)
        ones_col_r = perm.tile([128, 1], F32R)
        nc.vector.tensor_copy(ones_col_r[:], ones_col_f[:])
        ones_row_f = perm.tile([1, 128], F32)
        nc.vector.memset(ones_row_f[:], 1.0)
        ones_row_r = perm.tile([1, 128], F32R)
        nc.vector.tensor_copy(ones_row_r[:], ones_row_f[:])
        ones_row_b = perm.tile([1, 128], BF16)
        nc.vector.tensor_copy(ones_row_b[:], ones_row_f[:])
        ones_tok = perm.tile([1, TPC], BF16)
        nc.vector.memset(ones_tok[:], 1.0)

        def load_const(t_in, shape, tag, dt=F32):
            t = perm.tile(shape, dt, tag=tag)
            nc.sync.dma_start(t[:], t_in.ap())
            return t

        sqkv = load_const(sqkv_in, [1, 256], "c_sqkv", F32R)
        sv_row = perm.tile([1, 128], BF16, tag="c_sv")
        nc.sync.dma_start(sv_row[:], svb_in.ap()[0:1])
        bv_row = perm.tile([1, 128], BF16, tag="c_bv")
        nc.sync.dma_start(bv_row[:], svb_in.ap()[1:2])
        bqkr = load_const(bqkr_in, [1, 256], "c_bqkr", F32R)
        bo_r = load_const(bo_in, [1, D], "c_bo", BF16)      # 16*b_o
        bf1 = load_const(bf1_in, [128, M1], "c_bf1")        # 16*b_ff1
        bf1s = load_const(bf1s_in, [128, M1], "c_bf1s")     # SIGC/16 bias
        bf2_r = load_const(bf2_in, [1, D], "c_bf2", BF16)   # 256*b_ff2

        # ---- persistent SBUF ----
        invc = big.tile([128, NKT], F32R, tag="invc")  # inv keys-major
        invc8 = big.tile([128, NKT], F32R, tag="invc8")  # 0.125*inv (exp)
        invc16 = big.tile([128, NKT], F32R, tag="invc16")  # inv/16 (V evac)
        X2 = big.tile([128, PO, TPC], F32R, tag="x2")
        xh2 = big.tile([128, PO, TPC], FP8, tag="xh2")
        Amat = big.tile([128, M1, TPC], FP8, tag="amat")

        # dram scratch
        st_out = dram.tile([2, TPC], F32R)
        st_agg = dram.tile([NC_N, 2, TPC], F32R, addr_space="Shared")
        a2ai0 = dram.tile([NC_N, 64, TPC], FP8)
        a2ao0 = dram.tile([NC_N, 64, TPC], FP8)
        a2ai1 = dram.tile([NC_N, 64, TPC], FP8)
        a2ao1 = dram.tile([NC_N, 64, TPC], FP8)

        xb_view = xb_in.ap().rearrange("(po p) (k t) -> k p po t",
                                       p=128, t=TPC)

        xlp_cm = tc.tile_pool(name="xlp", bufs=1)
        xlp = xlp_cm.__enter__()
        xl = xlp.tile([128, PO, TPC], F32, tag="xl")   # my x (residual)
        wo_sb = xlp.tile([128, PO, 2 * D], FP8, tag="wo")
        qkvp_cm = tc.tile_pool(name="qkvp", bufs=1)
        qkvp = qkvp_cm.__enter__()
        Qh = qkvp.tile([128, BT], BF16, tag="qh")      # 2 heads stacked
        Kh = qkvp.tile([128, BT], BF16, tag="kh")
        Vt = qkvp.tile([128, HPC, NKT, 65], BF16, tag="vt")  # keys-major V
        nc.gpsimd.memset(Vt[:, :, :, 64:65], 1.0)   # softmax denominator row
        tri = perm.tile([128, 128], BF16, tag="tri")  # tri[p,q]=1 iff q>=p
        nc.gpsimd.memset(tri[:], 1.0)
        nc.gpsimd.affine_select(
            out=tri[:], in_=tri[:], compare_op=ALU.is_ge, fill=0.0,
            base=0, pattern=[[1, 128]], channel_multiplier=-1)

        # ============ Phase 1: local LN1 stats + AllGather ============
        with tc.tile_pool(name="wq", bufs=1) as wq, \
             tc.tile_pool(name="xsp", bufs=1) as xsp, \
             tc.tile_pool(name="xcp", bufs=2) as xcp, \
             tc.tile_pool(name="sqp", bufs=2) as sqp, \
             tc.tile_pool(name="strp", bufs=2) as strp, \
             tc.tile_pool(name="psA", bufs=6, space="PSUM") as psA, \
             tc.tile_pool(name="psV", bufs=2, space="PSUM") as psVt:
            xst = xsp.tile([128, PO, TPC], BF16)
            nc.sync.dma_start(
                xst[:], xs_in.ap().rearrange("(po p) t -> p po t", p=128))
            wqkv_sb = wq.tile([128, PO, 768], FP8)
            nc.sync.dma_start(
                wqkv_sb[:], wqkv_in.ap().rearrange("(po p) m -> p po m", p=128))
            xc0 = xcp.tile([128, PO, TPC], FP8, tag="xc")
            nc.sync.dma_start(xc0[:], xb_view[0])

            # stats on my 512 tokens
            stmu_l = rows.tile([1, TPC], F32R, tag="stmu_l")
            stinv_l = rows.tile([1, TPC], F32R, tag="stinv_l")
            ps_s = psA.tile([128, TPC], F32, tag="ps")
            for po in range(PO):
                nc.tensor.matmul(ps_s[0:1, :], ones_col_b[:], xst[:, po, :],
                                 start=(po == 0), stop=(po == PO - 1))
            ps_q = psA.tile([128, TPC], F32, tag="ps")
            for po in range(PO):
                sq = sqp.tile([128, TPC], BF16, tag="sq")
                nc.vector.tensor_mul(sq[:], xst[:, po, :], xst[:, po, :])
                nc.tensor.matmul(ps_q[0:1, :], ones_col_b[:], sq[:],
                                 start=(po == 0), stop=(po == PO - 1))
            nc.scalar.activation(stmu_l[:].bitcast(F32), ps_s[0:1, :],
                                 AF.Copy, scale=-1.0 / D)     # -mu
            ex2 = rows.tile([1, TPC], F32, tag="ex2")
            nc.scalar.activation(ex2[:], ps_q[0:1, :], AF.Copy, scale=1.0 / D)
            mu2 = rows.tile([1, TPC], F32, tag="mu2")
            nc.vector.tensor_mul(mu2[:], stmu_l[:].bitcast(F32),
                                 stmu_l[:].bitcast(F32))
            var = rows.tile([1, TPC], F32, tag="var")
            nc.vector.scalar_tensor_tensor(
                out=var[:], in0=ex2[:], scalar=EPS, in1=mu2[:],
                op0=ALU.add, op1=ALU.subtract)
            rec = rows.tile([1, TPC], F32, tag="rec")
            nc.vector.reciprocal(rec[:], var[:])
            nc.scalar.activation(stinv_l[:].bitcast(F32), rec[:],
                                 AF.Sqrt)                      # inv = 1/std
            nc.sync.dma_start(st_out[0:1, :], stmu_l[:])
            nc.sync.dma_start(st_out[1:2, :], stinv_l[:])
            nc.gpsimd.collective_compute(
                "AllGather", ALU.bypass, replica_groups=RG,
                ins=[st_out[:].opt()], outs=[st_agg[:].opt()])
            # gathered stats (ACT dma queue: don't block SP chunk stream)
            st_agg_v = st_agg[:].rearrange("s r t -> r s t")
            # inv keys-major + 0.125*inv for the exp scale (K's LN fold)
            st_agg_c = st_agg[:].rearrange("s r (c p) -> s r p c", p=128)
            for s in range(NC_N):
                nc.gpsimd.dma_start(invc[:, 4 * s:4 * s + 4], st_agg_c[s, 1])
            nc.vector.tensor_scalar_mul(invc8[:].bitcast(F32),
                                        invc[:].bitcast(F32), 0.125)
            nc.vector.tensor_scalar_mul(invc16[:].bitcast(F32),
                                        invc[:].bitcast(F32), 1.0 / 16.0)

            # ===== Phase 2: QKV, one fused pass =====
            # raw-G DoubleRow matmuls carry no stats dependency; the rank-1
            # LN corrections (s (x) -mu, b (x) std) append to each psum
            # group once the 48KB stats AllGather lands.
            for c in range(NC_N):
                if c == 0:
                    xc = xc0
                else:
                    xc = xcp.tile([128, PO, TPC], FP8, tag="xc")
                    nc.sync.dma_start(xc[:], xb_view[c])
                tok = slice(TPC * c, TPC * (c + 1))
                murow = strp.tile([1, TPC], F32R, tag="mur")
                nc.gpsimd.dma_start(murow[:], st_agg_v[0:1, c])
                invrow = strp.tile([1, TPC], F32R, tag="ivr")
                nc.sync.dma_start(invrow[:], st_agg_v[1:2, c])
                stdrow = strp.tile([1, TPC], F32R, tag="std")
                nc.vector.reciprocal(stdrow[:].bitcast(F32),
                                     invrow[:].bitcast(F32))
                mub = strp.tile([1, TPC], BF16, tag="mub")
                nc.vector.tensor_copy(mub[:], murow[:])
                stdb = strp.tile([1, TPC], BF16, tag="stdb")
                nc.vector.tensor_copy(stdb[:], stdrow[:])
                invsb = strp.tile([128, TPC], F32, tag="invsb")
                qk_ps = []
                for m in range(2):
                    ps = psA.tile([128, TPC], F32, tag="ps")
                    chi = slice(128 * m, 128 * m + 128)
                    clo = slice(384 + 128 * m, 384 + 128 * m + 128)
                    for k in range(PO // 2):
                        ksl = slice(2 * k, 2 * k + 2)
                        nc.tensor.matmul(ps[:], wqkv_sb[:, ksl, chi],
                                         xc[:, ksl, :], start=(k == 0),
                                         stop=False, perf_mode=DR)
                        nc.tensor.matmul(ps[:], wqkv_sb[:, ksl, clo],
                                         xc[:, ksl, :], start=False,
                                         stop=False, perf_mode=DR)
                    qk_ps.append(ps)
                vt_ps = []
                for kt in range(4):   # V in keys-major layout
                    vsl = slice(128 * kt, 128 * kt + 128)
                    psv = psVt.tile([128, 128], F32, tag="psv")
                    for k in range(PO // 2):
                        ksl = slice(2 * k, 2 * k + 2)
                        nc.tensor.matmul(
                            psv[:], xc[:, ksl, vsl],
                            wqkv_sb[:, ksl, 256:384], start=(k == 0),
                            stop=False, perf_mode=DR)
                        nc.tensor.matmul(
                            psv[:], xc[:, ksl, vsl],
                            wqkv_sb[:, ksl, 640:768], start=False,
                            stop=False, perf_mode=DR)
                    vt_ps.append(psv)
                # stats-dependent tail: inv broadcast + rank-1 corrections
                psb = psA.tile([128, TPC], F32, tag="ps")
                nc.tensor.matmul(psb[:], ones_row_r[:], invrow[:],
                                 start=True, stop=True)
                nc.scalar.activation(invsb[:], psb[:], AF.Copy)
                for m in range(2):
                    ps = qk_ps[m]
                    # ps holds 16*G: corrections are scaled by 16 to match
                    nc.tensor.matmul(ps[:], sqkv[:, 128 * m:128 * m + 128],
                                     murow[:], start=False, stop=False)
                    nc.tensor.matmul(ps[:], bqkr[:, 128 * m:128 * m + 128],
                                     stdrow[:], start=False, stop=True)
                nc.vector.scalar_tensor_tensor(
                    out=Qh[:, tok], in0=qk_ps[0][:], scalar=1.0 / 16.0,
                    in1=invsb[:], op0=ALU.mult, op1=ALU.mult)
                nc.vector.tensor_scalar_mul(Kh[:, tok], qk_ps[1][:],
                                            1.0 / 16.0)
                for kt in range(4):
                    g = 4 * c + kt
                    psv = vt_ps[kt]
                    ksl = slice(128 * kt, 128 * kt + 128)
                    nc.tensor.matmul(psv[:], mub[:, ksl], sv_row[:],
                                     start=False, stop=False)
                    nc.tensor.matmul(psv[:], stdb[:, ksl], bv_row[:],
                                     start=False, stop=True)
                    for h in range(HPC):
                        nc.vector.tensor_scalar(
                            out=Vt[:, h, g, 0:64],
                            in0=psv[:, 64 * h:64 * h + 64],
                            scalar1=1.0 / 16.0,
                            scalar2=invc[:, g:g + 1].bitcast(F32),
                            op0=ALU.mult, op1=ALU.mult)

            # residual x + Wo weights: needed from phase 4 on
            nc.sync.dma_start(xl[:],
                              xf_in.ap().rearrange("(po p) t -> p po t", p=128))
            nc.sync.dma_start(
                wo_sb[:], wo_in.ap().rearrange("(po p) n -> p po n", p=128))

        # ============ Phase 3: causal attention ============
        with tc.tile_pool(name="ptp", bufs=4) as ptp, \
             tc.tile_pool(name="rcd", bufs=2) as rcdp, \
             tc.tile_pool(name="avp", bufs=2) as avp, \
             tc.tile_pool(name="psS", bufs=2, space="PSUM") as psS, \
             tc.tile_pool(name="psV2", bufs=2, space="PSUM") as psV2:
            pending = []
            a2a_pairs = [(a2ai0, a2ao0), (a2ai1, a2ao1)]
            for h in range(HPC):
                hsl = slice(64 * h, 64 * h + 64)
                for b in range(B):
                    for qh in range(2):
                        n_i = 8 if qh == 0 else 16
                        ps_av = psV2.tile([65, 1024], F32, tag="psav")
                        for i in range(n_i):
                            if i == 2 and pending:
                                pending.pop()()
                            koff = 2048 * b + 128 * i
                            qlo = max(1024 * qh, 128 * i)
                            free = 1024 * (qh + 1) - qlo
                            off = qlo - 1024 * qh
                            qabs = 2048 * b + qlo
                            # matmul outputs must stay inside one psum bank
                            # (512 f32): emit per-bank-half pieces, aligned
                            # to each psum tile's own banks.
                            pieces = []   # ps_av-aligned (offset off..1024)
                            lo = off
                            while lo < 1024:
                                hi = min(1024, (lo // 512 + 1) * 512)
                                pieces.append((lo, hi - lo))
                                lo = hi
                            spieces = []  # pss-aligned (offset 0..free)
                            lo = 0
                            while lo < free:
                                hi = min(free, (lo // 512 + 1) * 512)
                                spieces.append((lo, hi - lo))
                                lo = hi
                            pss = psS.tile([128, 1024], F32, tag="pss")
                            for (plo, pfree) in spieces:
                                qa = qabs + plo
                                nc.tensor.matmul(
                                    pss[:, plo:plo + pfree],
                                    Kh[hsl, koff:koff + 128],
                                    Qh[hsl, qa:qa + pfree],
                                    start=True, stop=True)
                            pt = ptp.tile([128, 1024], BF16, tag="pt")
                            nc.scalar.activation(
                                pt[:, 0:free], pss[:, 0:free], AF.Exp,
                                scale=invc8[:, 16 * b + i:16 * b + i + 1]
                                .bitcast(F32))
                            if 128 * i >= 1024 * qh:   # diagonal tile
                                if h == 0:   # Pool is free before A2A#0
                                    nc.gpsimd.affine_select(
                                        out=pt[:, 0:128], in_=pt[:, 0:128],
                                        compare_op=ALU.is_ge, fill=0.0,
                                        base=0, pattern=[[1, 128]],
                                        channel_multiplier=-1)
                                else:        # Pool busy with A2A#0
                                    nc.vector.tensor_mul(pt[:, 0:128],
                                                         pt[:, 0:128],
                                                         tri[:])
                            for (plo, pfree) in pieces:
                                hb = plo // 512
                                last_i = 8 * qh + 3 if hb == 0 else n_i - 1
                                nc.tensor.matmul(
                                    ps_av[:, plo:plo + pfree],
                                    Vt[:, h, 16 * b + i, :],
                                    pt[:, plo - off:plo - off + pfree],
                                    start=(i == 0), stop=(i == last_i))
                        # epilogue: copy to SBUF + recip on DVE; the PE
                        # broadcast is deferred into the next group's loop
                        # so it doesn't head-of-line block the next scores.
                        avs = avp.tile([65, 1024], F32, tag="avs")
                        nc.vector.tensor_copy(avs[:], ps_av[:])
                        recd = rcdp.tile([1, 1024], F32R, tag="recd")
                        nc.vector.reciprocal(recd[:].bitcast(F32),
                                             avs[64:65, :])

                        def epilogue(avs=avs, recd=recd, b=b, qh=qh,
                                     h=h):
                            ps_bc = psS.tile([128, 1024], F32, tag="pss")
                            for half in range(2):
                                nc.tensor.matmul(
                                    ps_bc[0:64, 512 * half:512 * half + 512],
                                    ones_row_r[:, 0:64],
                                    recd[:, 512 * half:512 * half + 512],
                                    start=True, stop=True)
                            avn = avp.tile([64, 1024], FP8, tag="avn")
                            nc.vector.tensor_mul(avn[:], avs[0:64, :],
                                                 ps_bc[0:64, :])
                            g0 = 4 * b + 2 * qh
                            a2aih = a2a_pairs[h][0]
                            nc.sync.dma_start(a2aih[g0, :, :], avn[:, 0:TPC])
                            nc.sync.dma_start(a2aih[g0 + 1, :, :],
                                              avn[:, TPC:2 * TPC])
                        pending.append(epilogue)

                if b == B - 1:      # this head-half is complete
                    while pending:
                        pending.pop()()
                    if h == 0:      # launch early; h=1's goes after the
                        nc.gpsimd.collective_compute(   # pool scope closes
                            "AllToAll", ALU.bypass, replica_groups=RG,
                            ins=[a2ai0[:].opt()], outs=[a2ao0[:].opt()])

        qkvp_cm.__exit__(None, None, None)   # free Qh/Kh/Vt

        # ==== Phases 4-6 in one pool scope: everything here is emitted
        # before the second AllToAll so pool-open barriers don't serialize
        # on it; Wo's first half + weight prefetches overlap the collective.
        with tc.tile_pool(name="avtp", bufs=1) as avtp, \
             tc.tile_pool(name="w1p", bufs=2) as w1p, \
             tc.tile_pool(name="w2p", bufs=2) as w2p, \
             tc.tile_pool(name="tmp2", bufs=2) as tmp2p, \
             tc.tile_pool(name="sgp", bufs=2) as sgp, \
             tc.tile_pool(name="outp", bufs=2) as outp, \
             tc.tile_pool(name="psA2", bufs=8, space="PSUM") as psA2:
            psB = psA2
            w1_sb = []
            w2_sb = []
            for g in range(4):   # prefetch during the A2A / Wo phase
                w1t = w1p.tile([128, PO, 2048], FP8, tag="w1")
                nc.sync.dma_start(
                    w1t[:, :, 0:1024], wf1_in.ap()[:, 1024 * g:1024 * (g + 1)]
                    .rearrange("(po p) n -> p po n", p=128))
                nc.sync.dma_start(
                    w1t[:, :, 1024:2048],
                    wf1_in.ap()[:, DFF + 1024 * g:DFF + 1024 * (g + 1)]
                    .rearrange("(po p) n -> p po n", p=128))
                w1_sb.append(w1t)
            for g in range(4):
                w2t = w2p.tile([128, M1, 512], FP8, tag="w2")
                nc.sync.dma_start(
                    w2t[:], wf2_in.ap()[:, 512 * g:512 * (g + 1)]
                    .rearrange("(ko p) n -> p ko n", p=128))
                w2_sb.append(w2t)

            AVt0 = avtp.tile([128, 4, TPC], FP8, tag="avt0")
            nc.scalar.dma_start(
                AVt0[:],
                a2ao0[:].rearrange("(po s2) p t -> (s2 p) po t", s2=2))
            wo_ps = []
            for m in range(PO):   # h0 half: overlaps the second AllToAll
                ps = psA2.tile([128, TPC], F32, tag="ps")
                chi = slice(128 * m, 128 * m + 128)
                clo = slice(D + 128 * m, D + 128 * m + 128)
                for k in range(2):
                    ksl = slice(2 * k, 2 * k + 2)
                    nc.tensor.matmul(ps[:], wo_sb[:, ksl, chi],
                                     AVt0[:, ksl, :], start=(k == 0),
                                     stop=False, perf_mode=DR)
                    nc.tensor.matmul(ps[:], wo_sb[:, ksl, clo],
                                     AVt0[:, ksl, :], start=False,
                                     stop=False, perf_mode=DR)
                wo_ps.append(ps)

            nc.gpsimd.collective_compute(
                "AllToAll", ALU.bypass, replica_groups=RG,
                ins=[a2ai1[:].opt()], outs=[a2ao1[:].opt()])
            AVt1 = avtp.tile([128, 4, TPC], FP8, tag="avt1")
            nc.scalar.dma_start(
                AVt1[:],
                a2ao1[:].rearrange("(po s2) p t -> (s2 p) po t", s2=2))

            for m in range(PO):   # h1 half + epilogue
                ps = wo_ps[m]
                chi = slice(128 * m, 128 * m + 128)
                clo = slice(D + 128 * m, D + 128 * m + 128)
                for k in range(2):
                    ksl = slice(2 * k, 2 * k + 2)
                    nc.tensor.matmul(ps[:], wo_sb[:, 4 + 2 * k:6 + 2 * k, chi],
                                     AVt1[:, ksl, :], start=False,
                                     stop=False, perf_mode=DR)
                    nc.tensor.matmul(ps[:], wo_sb[:, 4 + 2 * k:6 + 2 * k, clo],
                                     AVt1[:, ksl, :], start=False,
                                     stop=False, perf_mode=DR)
                nc.tensor.matmul(ps[:], bo_r[:, 128 * m:128 * m + 128],
                                 ones_tok[:], start=False, stop=True)
                nc.vector.scalar_tensor_tensor(
                    out=X2[:, m, :].bitcast(F32), in0=ps[:],
                    scalar=1.0 / 16.0, in1=xl[:, m, :],
                    op0=ALU.mult, op1=ALU.add)

            ps_s2 = psB.tile([128, TPC], F32, tag="ps")
            for po in range(PO):
                nc.tensor.matmul(ps_s2[0:1, :], ones_col_r[:],
                                 X2[:, po, :],
                                 start=(po == 0), stop=(po == PO - 1))
            ps_q2 = psB.tile([128, TPC], F32, tag="ps")
            for po in range(PO):
                sq = tmp2p.tile([128, TPC], F32R, tag="sq2")
                nc.scalar.activation(sq[:].bitcast(F32),
                                     X2[:, po, :].bitcast(F32), AF.Square)
                nc.tensor.matmul(ps_q2[0:1, :], ones_col_r[:], sq[:],
                                 start=(po == 0), stop=(po == PO - 1))
            nmu2 = rows.tile([1, TPC], F32R, tag="nmu2")
            nc.scalar.activation(nmu2[:].bitcast(F32), ps_s2[0:1, :],
                                 AF.Copy, scale=-1.0 / D)
            ex2b = rows.tile([1, TPC], F32, tag="ex2b")
            nc.scalar.activation(ex2b[:], ps_q2[0:1, :], AF.Copy,
                                 scale=1.0 / D)
            mu2b = rows.tile([1, TPC], F32, tag="mu2b")
            nc.vector.tensor_mul(mu2b[:], nmu2[:].bitcast(F32),
                                 nmu2[:].bitcast(F32))
            varb = rows.tile([1, TPC], F32, tag="varb")
            nc.vector.scalar_tensor_tensor(
                out=varb[:], in0=ex2b[:], scalar=EPS, in1=mu2b[:],
                op0=ALU.add, op1=ALU.subtract)
            recb = rows.tile([1, TPC], F32, tag="recb")
            nc.vector.reciprocal(recb[:], varb[:])
            inv2 = rows.tile([1, TPC], F32R, tag="inv2")
            nc.scalar.activation(inv2[:].bitcast(F32), recb[:], AF.Sqrt)
            ps_mu = psB.tile([128, TPC], F32, tag="ps")
            nc.tensor.matmul(ps_mu[:], ones_row_r[:], nmu2[:],
                             start=True, stop=True)
            ps_iv = psB.tile([128, TPC], F32, tag="ps")
            nc.tensor.matmul(ps_iv[:], ones_row_r[:], inv2[:],
                             start=True, stop=True)
            for po in range(PO):
                t0 = tmp2p.tile([128, TPC], F32, tag="t0")
                nc.vector.tensor_add(t0[:], X2[:, po, :].bitcast(F32),
                                     ps_mu[:])
                nc.vector.tensor_mul(xh2[:, po, :], t0[:], ps_iv[:])

            for m in range(M1):
                ps = psB.tile([128, TPC], F32, tag="ps")
                w1t = w1_sb[m // 8]
                chi = slice(128 * (m % 8), 128 * (m % 8) + 128)
                clo = slice(1024 + 128 * (m % 8), 1024 + 128 * (m % 8) + 128)
                for k in range(PO // 2):
                    ksl = slice(2 * k, 2 * k + 2)
                    nc.tensor.matmul(ps[:], w1t[:, ksl, chi],
                                     xh2[:, ksl, :], start=(k == 0),
                                     stop=False, perf_mode=DR)
                    nc.tensor.matmul(ps[:], w1t[:, ksl, clo],
                                     xh2[:, ksl, :], start=False,
                                     stop=(k == PO // 2 - 1), perf_mode=DR)
                sg = sgp.tile([128, TPC], BF16, tag="sg")
                nc.scalar.activation(sg[:], ps[:], AF.Sigmoid,
                                     scale=SIGC / 16.0,
                                     bias=bf1s[:, m:m + 1])
                nc.vector.scalar_tensor_tensor(
                    out=Amat[:, m, :], in0=ps[:],
                    scalar=bf1[:, m:m + 1], in1=sg[:],
                    op0=ALU.add, op1=ALU.mult)
            out_view = out_t.ap().rearrange("(po p) t -> p po t", p=128)
            for m in range(PO):
                ps = psB.tile([128, TPC], F32, tag="ps")
                w2t = w2_sb[m // 2]
                chi = slice(128 * (m % 2), 128 * (m % 2) + 128)
                clo = slice(256 + 128 * (m % 2), 256 + 128 * (m % 2) + 128)
                for k in range(M1 // 2):
                    ksl = slice(2 * k, 2 * k + 2)
                    nc.tensor.matmul(ps[:], w2t[:, ksl, chi],
                                     Amat[:, ksl, :], start=(k == 0),
                                     stop=False, perf_mode=DR)
                    nc.tensor.matmul(ps[:], w2t[:, ksl, clo],
                                     Amat[:, ksl, :], start=False,
                                     stop=False, perf_mode=DR)
                nc.tensor.matmul(ps[:], bf2_r[:, 128 * m:128 * m + 128],
                                 ones_tok[:], start=False, stop=True)
                om = outp.tile([128, TPC], F32, tag="om")
                nc.vector.scalar_tensor_tensor(
                    out=om[:], in0=ps[:], scalar=1.0 / 256.0,
                    in1=X2[:, m, :].bitcast(F32),
                    op0=ALU.mult, op1=ALU.add)
                nc.sync.dma_start(out_view[:, m, :], om[:])
        xlp_cm.__exit__(None, None, None)    # free xl/wo_sb

    nc.compile()
    return nc


def _get_nc():
    key = ("nc", GELU_NATIVE)
    if key not in _CACHE:
        _CACHE[key] = _build()
    return _CACHE[key]


def _make_in_maps(inputs):
    x = np.asarray(inputs["x"], np.float32).reshape(BT, D)
    ln1w = np.asarray(inputs["ln1_w"], np.float32)
    ln1b = np.asarray(inputs["ln1_b"], np.float32)
    ln2w = np.asarray(inputs["ln2_w"], np.float32)
    ln2b = np.asarray(inputs["ln2_b"], np.float32)
    W_qkv0 = np.asarray(inputs["W_qkv"], np.float32)
    W_qkv = W_qkv0 * ln1w[:, None]
    b_qkv = np.asarray(inputs["b_qkv"], np.float32) + ln1b @ W_qkv0
    W_o = np.asarray(inputs["W_o"], np.float32)
    b_o = np.asarray(inputs["b_o"], np.float32)
    W_ff10 = np.asarray(inputs["W_ff1"], np.float32)
    W_ff1 = W_ff10 * ln2w[:, None]
    b_ff1 = np.asarray(inputs["b_ff1"], np.float32) + ln2b @ W_ff10
    W_ff2 = np.asarray(inputs["W_ff2"], np.float32)
    b_ff2 = np.asarray(inputs["b_ff2"], np.float32)

    def pcol(v):  # [D'] -> [128, D'/128] per-partition column layout
        return np.ascontiguousarray(v.reshape(-1, 128).T)

    xT = np.ascontiguousarray(x.T)                      # [D, BT] f32

    def hilo16(w):   # 16*w as fp8 hi + fp8 residual, concatenated wide
        base = (16.0 * w).astype(np.float32)
        hi = base.astype(NPFP8)
        lo = (base - hi.astype(np.float32)).astype(NPFP8)
        return np.ascontiguousarray(np.concatenate([hi, lo], axis=1))

    def hilo16_blocked(w, blk):   # [hi0|lo0|hi1|lo1|...] per blk columns
        base = (16.0 * w).astype(np.float32)
        hi = base.astype(NPFP8)
        lo = (base - hi.astype(np.float32)).astype(NPFP8)
        parts = []
        for g in range(w.shape[1] // blk):
            parts.append(hi[:, blk * g:blk * (g + 1)])
            parts.append(lo[:, blk * g:blk * (g + 1)])
        return np.ascontiguousarray(np.concatenate(parts, axis=1))

    common = {
        "xb": xT.astype(NPFP8),
        "wo": hilo16(W_o[WO_PERM]),
        "bo": np.ascontiguousarray(16.0 * b_o).reshape(1, D).astype(NPBF16),
        "wf1": hilo16(W_ff1),
        "bf1": pcol(16.0 * b_ff1),
        "bf1s": pcol(SIGC * b_ff1).astype(np.float32),
        "wf2": hilo16_blocked(W_ff2, 256),
        "bf2": np.ascontiguousarray(
            256.0 * b_ff2).reshape(1, D).astype(NPBF16),
    }
    in_maps = []
    for r in range(NC_N):
        hc = 128 * r          # first column of this core's Q/K/V head block
        m = dict(common)
        m["xs"] = np.ascontiguousarray(
            xT[:, TPC * r:TPC * (r + 1)]).astype(NPBF16)
        m["xf"] = np.ascontiguousarray(xT[:, TPC * r:TPC * (r + 1)])
        wq = W_qkv[:, hc:hc + 128]
        wk = W_qkv[:, D + hc:D + hc + 128]
        wv = W_qkv[:, 2 * D + hc:2 * D + hc + 128]
        m["wqkv"] = hilo16(np.concatenate([wq, wk, wv], axis=1))
        wdq = (m["wqkv"][:, 0:384].astype(np.float32)
               + m["wqkv"][:, 384:768].astype(np.float32))   # = 16*W'
        m["sqkv"] = np.ascontiguousarray(
            wdq[:, 0:256].sum(0).reshape(1, 256)).astype(np.float32)
        m["svb"] = np.ascontiguousarray(np.stack(
            [wdq[:, 256:384].sum(0),
             16.0 * b_qkv[2 * D + hc:2 * D + hc + 128]])).astype(NPBF16)
        m["bqkr"] = np.ascontiguousarray(16.0 * np.concatenate(
            [b_qkv[hc:hc + 128], b_qkv[D + hc:D + hc + 128]])
            .reshape(1, 256)).astype(np.float32)
        in_maps.append(m)
    return in_maps


def _run_sim(nc, in_maps):
    """Instruction-level simulator fallback executor (same program)."""
    from concourse.bass_interp import MultiCoreSim
    sim = MultiCoreSim(nc, num_cores=NC_N, require_finite=False)
    for i in range(NC_N):
        for k, v in in_maps[i].items():
            sim.cores[i].tensor(k)[:] = np.asarray(v)
    sim.simulate(check_with_hw=False)
    return [np.array(sim.cores[i].tensor("outt")) for i in range(NC_N)]


def _run(inputs, trace=False, trace_cores=None):
    nc = _get_nc()
    in_maps = _make_in_maps(inputs)
    res = None
    try:
        res = bass_utils.run_bass_kernel_spmd(
            nc, in_maps, core_ids=list(range(NC_N)), trace=trace,
            trace_cores=trace_cores)
        outs = [res.results[r]["outt"] for r in range(NC_N)]
    except Exception:
        outs = _run_sim(nc, in_maps)
    full = np.concatenate([np.asarray(o, np.float32).T for o in outs], axis=0)
    return full.reshape(B, T, D).astype(np.float32), res


def kernel(**inputs):
    out, _ = _run(inputs, trace=False)
    return out


# revision 62
# speedup vs baseline: 1.0030x; 1.0030x over previous
"""Trainium2 Bass kernel for a dense transformer block (nn_Block_88338887344891).

Distribution over 8 NeuronCores (single SPMD NEFF, 2 collectives):
  - LN1 stats computed per-core on its own 512 tokens, AllGathered (48KB,
    ~16us, overlapped with the QKV matmuls on raw x).
  - LayerNorm is folded into the matmuls: QKV = inv * (W'^T x + s (-mu)) + b'
    where W' = diag(ln_w) W (host-folded), s = colsum(W'), so the heavy
    matmuls start before the stats arrive; ln_b is folded into b'.
  - QKV + causal attention head-sharded (2 heads/core over all 4096 tokens);
    scores/exp/AV trimmed to the causal region; V is produced directly in
    keys-major layout (x as the stationary operand) so no transposes.
  - attention output AllToAll per head in bf16 (1MB, ~41us)
  - output projection + residual + LN2 + full MLP token-sharded (512 tok/core)
  - gelu(tanh approx) via t*sigmoid(1.702 t) (1 ACT + 1 fused DVE op)
All matmul moving operands are bf16 (1 cycle/row); PSUM accumulates f32.
"""
import numpy as np
from contextlib import ExitStack

try:  # persistent XLA cache so repeat runs skip the NEFF compile
    import jax
    jax.config.update("jax_compilation_cache_dir", "/tmp/jax_neff_cache")
    jax.config.update("jax_persistent_cache_min_compile_time_secs", 1.0)
except Exception:
    pass

import ml_dtypes
import concourse.bass as bass
import concourse.bacc as bacc
import concourse.tile as tile
import concourse.mybir as mybir
from concourse import bass_utils

AF = mybir.ActivationFunctionType
ALU = mybir.AluOpType
F32 = mybir.dt.float32
F32R = mybir.dt.float32r
BF16 = mybir.dt.bfloat16
FP8 = mybir.dt.float8e4
NPBF16 = ml_dtypes.bfloat16
NPFP8 = mybir.dt.np(mybir.dt.float8e4)
DR = mybir.MatmulPerfMode.DoubleRow

NC_N = 8          # cores
B, T, D, H = 2, 2048, 1024, 16
HD = D // H       # 64
DFF = 4 * D       # 4096
EPS = 1e-5
BT = B * T               # 4096 tokens
TPC = BT // NC_N         # 512 tokens per core
HPC = H // NC_N          # 2 heads per core
PO = D // 128            # 8 D-tiles
M1 = DFF // 128          # 32 ff1 out tiles
NKT = BT // 128          # 32 key tiles globally (16 per batch)
RG = [list(range(NC_N))]
SIGC = 1.702             # gelu sigmoid-form constant

GELU_NATIVE = False   # kept for test.py compat; kernel is sim/hw identical

# Wo row order after the per-head AllToAll halves: feature index
# n = half*512 + po*128 + s2*64 + d maps to old row 64*(2*(2*po+s2)+half)+d
WO_PERM = np.array([64 * (2 * (2 * po + s2) + half) + d
                    for half in range(2) for po in range(4)
                    for s2 in range(2) for d in range(64)])

_CACHE = {}


def _build():
    nc = bacc.Bacc("TRN2", target_bir_lowering=False, debug=False,
                   num_devices=NC_N)

    # ---- per-core external inputs ----
    xb_in = nc.dram_tensor("xb", [D, BT], FP8, kind="ExternalInput")
    xs_in = nc.dram_tensor("xs", [D, TPC], BF16, kind="ExternalInput")
    xf_in = nc.dram_tensor("xf", [D, TPC], F32, kind="ExternalInput")
    wqkv_in = nc.dram_tensor("wqkv", [D, 768], FP8, kind="ExternalInput")
    sqkv_in = nc.dram_tensor("sqkv", [1, 256], F32R, kind="ExternalInput")
    svb_in = nc.dram_tensor("svb", [2, 128], BF16, kind="ExternalInput")
    bqkr_in = nc.dram_tensor("bqkr", [1, 256], F32R, kind="ExternalInput")
    wo_in = nc.dram_tensor("wo", [D, 2 * D], FP8, kind="ExternalInput")
    bo_in = nc.dram_tensor("bo", [1, D], BF16, kind="ExternalInput")
    wf1_in = nc.dram_tensor("wf1", [D, 2 * DFF], FP8, kind="ExternalInput")
    bf1_in = nc.dram_tensor("bf1", [128, M1], F32, kind="ExternalInput")
    bf1s_in = nc.dram_tensor("bf1s", [128, M1], F32, kind="ExternalInput")
    wf2_in = nc.dram_tensor("wf2", [DFF, 2 * D], FP8, kind="ExternalInput")
    bf2_in = nc.dram_tensor("bf2", [1, D], BF16, kind="ExternalInput")
    out_t = nc.dram_tensor("outt", [D, TPC], F32, kind="ExternalOutput")

    with tile.TileContext(nc, pool_alloc_mode="queue") as tc, \
            ExitStack() as ctx:
        perm = ctx.enter_context(tc.tile_pool(name="perm", bufs=1))
        big = ctx.enter_context(tc.tile_pool(name="big", bufs=1))
        rows = ctx.enter_context(tc.tile_pool(name="rows", bufs=1))
        dram = ctx.enter_context(tc.tile_pool(name="dram", bufs=1, space="DRAM"))

        # ---- constants ----
        ones_col_b = perm.tile([128, 1], BF16)
        nc.vector.memset(ones_col_b[:], 1.0)
        ones_col_f = perm.tile([128, 1], F32)
        nc.vector.memset(ones_col_f[:], 1.0)
        ones_col_r = perm.tile([128, 1], F32R)
        nc.vector.tensor_copy(ones_col_r[:], ones_col_f[:])
        ones_row_f = perm.tile([1, 128], F32)
        nc.vector.memset(ones_row_f[:], 1.0)
        ones_row_r = perm.tile([1, 128], F32R)
        nc.vector.tensor_copy(ones_row_r[:], ones_row_f[:])
        ones_row_b = perm.tile([1, 128], BF16)
        nc.vector.tensor_copy(ones_row_b[:], ones_row_f[:])
        ones_tok = perm.tile([1, TPC], BF16)
        nc.vector.memset(ones_tok[:], 1.0)

        def load_const(t_in, shape, tag, dt=F32):
            t = perm.tile(shape, dt, tag=tag)
            nc.sync.dma_start(t[:], t_in.ap())
            return t

        sqkv = load_const(sqkv_in, [1, 256], "c_sqkv", F32R)
        sv_row = perm.tile([1, 128], BF16, tag="c_sv")
        nc.sync.dma_start(sv_row[:], svb_in.ap()[0:1])
        bv_row = perm.tile([1, 128], BF16, tag="c_bv")
        nc.sync.dma_start(bv_row[:], svb_in.ap()[1:2])
        bqkr = load_const(bqkr_in, [1, 256], "c_bqkr", F32R)
        bo_r = load_const(bo_in, [1, D], "c_bo", BF16)      # 16*b_o
        bf1 = load_const(bf1_in, [128, M1], "c_bf1")        # 16*b_ff1
        bf1s = load_const(bf1s_in, [128, M1], "c_bf1s")     # SIGC/16 bias
        bf2_r = load_const(bf2_in, [1, D], "c_bf2", BF16)   # 256*b_ff2

        # ---- persistent SBUF ----
        invc = big.tile([128, NKT], F32R, tag="invc")  # inv keys-major
        invc8 = big.tile([128, NKT], F32R, tag="invc8")  # 0.125*inv (exp)
        invc16 = big.tile([128, NKT], F32R, tag="invc16")  # inv/16 (V evac)
        X2 = big.tile([128, PO, TPC], F32R, tag="x2")
        xh2 = big.tile([128, PO, TPC], FP8, tag="xh2")
        Amat = big.tile([128, M1, TPC], FP8, tag="amat")

        # dram scratch
        st_out = dram.tile([2, TPC], F32R)
        st_agg = dram.tile([NC_N, 2, TPC], F32R, addr_space="Shared")
        a2ai0 = dram.tile([NC_N, 64, TPC], FP8)
        a2ao0 = dram.tile([NC_N, 64, TPC], FP8)
        a2ai1 = dram.tile([NC_N, 64, TPC], FP8)
        a2ao1 = dram.tile([NC_N, 64, TPC], FP8)

        xb_view = xb_in.ap().rearrange("(po p) (k t) -> k p po t",
                                       p=128, t=TPC)

        xlp_cm = tc.tile_pool(name="xlp", bufs=1)
        xlp = xlp_cm.__enter__()
        xl = xlp.tile([128, PO, TPC], F32, tag="xl")   # my x (residual)
        wo_sb = xlp.tile([128, PO, 2 * D], FP8, tag="wo")
        qkvp_cm = tc.tile_pool(name="qkvp", bufs=1)
        qkvp = qkvp_cm.__enter__()
        Qh = qkvp.tile([128, BT], BF16, tag="qh")      # 2 heads stacked
        Kh = qkvp.tile([128, BT], BF16, tag="kh")
        Vt = qkvp.tile([128, HPC, NKT, 65], BF16, tag="vt")  # keys-major V
        nc.gpsimd.memset(Vt[:, :, :, 64:65], 1.0)   # softmax denominator row
        tri = perm.tile([128, 128], BF16, tag="tri")  # tri[p,q]=1 iff q>=p
        nc.gpsimd.memset(tri[:], 1.0)
        nc.gpsimd.affine_select(
            out=tri[:], in_=tri[:], compare_op=ALU.is_ge, fill=0.0,
            base=0, pattern=[[1, 128]], channel_multiplier=-1)

        # ============ Phase 1: local LN1 stats + AllGather ============
        with tc.tile_pool(name="wq", bufs=1) as wq, \
             tc.tile_pool(name="xsp", bufs=1) as xsp, \
             tc.tile_pool(name="xcp", bufs=2) as xcp, \
             tc.tile_pool(name="sqp", bufs=2) as sqp, \
             tc.tile_pool(name="strp", bufs=2) as strp, \
             tc.tile_pool(name="psA", bufs=4, space="PSUM") as psA, \
             tc.tile_pool(name="psBb", bufs=2, space="PSUM") as psbp, \
             tc.tile_pool(name="psV", bufs=2, space="PSUM") as psVt:
            xst = xsp.tile([128, PO, TPC], BF16)
            nc.sync.dma_start(
                xst[:], xs_in.ap().rearrange("(po p) t -> p po t", p=128))
            wqkv_sb = wq.tile([128, PO, 768], FP8)
            nc.sync.dma_start(
                wqkv_sb[:], wqkv_in.ap().rearrange("(po p) m -> p po m", p=128))
            xc0 = xcp.tile([128, PO, TPC], FP8, tag="xc")
            nc.sync.dma_start(xc0[:], xb_view[0])

            # stats on my 512 tokens
            stmu_l = rows.tile([1, TPC], F32R, tag="stmu_l")
            stinv_l = rows.tile([1, TPC], F32R, tag="stinv_l")
            ps_s = psA.tile([128, TPC], F32, tag="ps")
            for po in range(PO):
                nc.tensor.matmul(ps_s[0:1, :], ones_col_b[:], xst[:, po, :],
                                 start=(po == 0), stop=(po == PO - 1))
            ps_q = psA.tile([128, TPC], F32, tag="ps")
            for po in range(PO):
                sq = sqp.tile([128, TPC], BF16, tag="sq")
                nc.vector.tensor_mul(sq[:], xst[:, po, :], xst[:, po, :])
                nc.tensor.matmul(ps_q[0:1, :], ones_col_b[:], sq[:],
                                 start=(po == 0), stop=(po == PO - 1))
            nc.scalar.activation(stmu_l[:].bitcast(F32), ps_s[0:1, :],
                                 AF.Copy, scale=-1.0 / D)     # -mu
            ex2 = rows.tile([1, TPC], F32, tag="ex2")
            nc.scalar.activation(ex2[:], ps_q[0:1, :], AF.Copy, scale=1.0 / D)
            mu2 = rows.tile([1, TPC], F32, tag="mu2")
            nc.vector.tensor_mul(mu2[:], stmu_l[:].bitcast(F32),
                                 stmu_l[:].bitcast(F32))
            var = rows.tile([1, TPC], F32, tag="var")
            nc.vector.scalar_tensor_tensor(
                out=var[:], in0=ex2[:], scalar=EPS, in1=mu2[:],
                op0=ALU.add, op1=ALU.subtract)
            rec = rows.tile([1, TPC], F32, tag="rec")
            nc.vector.reciprocal(rec[:], var[:])
            nc.scalar.activation(stinv_l[:].bitcast(F32), rec[:],
                                 AF.Sqrt)                      # inv = 1/std
            nc.sync.dma_start(st_out[0:1, :], stmu_l[:])
            nc.sync.dma_start(st_out[1:2, :], stinv_l[:])
            nc.gpsimd.collective_compute(
                "AllGather", ALU.bypass, replica_groups=RG,
                ins=[st_out[:].opt()], outs=[st_agg[:].opt()])
            # gathered stats (ACT dma queue: don't block SP chunk stream)
            st_agg_v = st_agg[:].rearrange("s r t -> r s t")
            # inv keys-major + 0.125*inv for the exp scale (K's LN fold)
            st_agg_c = st_agg[:].rearrange("s r (c p) -> s r p c", p=128)
            for s in range(NC_N):
                nc.gpsimd.dma_start(invc[:, 4 * s:4 * s + 4], st_agg_c[s, 1])
            nc.vector.tensor_scalar_mul(invc8[:].bitcast(F32),
                                        invc[:].bitcast(F32), 0.125)
            nc.vector.tensor_scalar_mul(invc16[:].bitcast(F32),
                                        invc[:].bitcast(F32), 1.0 / 16.0)

            # ===== Phase 2: QKV, one fused pass =====
            # raw-G DoubleRow matmuls carry no stats dependency; the rank-1
            # LN corrections (s (x) -mu, b (x) std) append to each psum
            # group once the 48KB stats AllGather lands.
            for c in range(NC_N):
                if c == 0:
                    xc = xc0
                else:
                    xc = xcp.tile([128, PO, TPC], FP8, tag="xc")
                    nc.sync.dma_start(xc[:], xb_view[c])
                tok = slice(TPC * c, TPC * (c + 1))
                murow = strp.tile([1, TPC], F32R, tag="mur")
                nc.gpsimd.dma_start(murow[:], st_agg_v[0:1, c])
                invrow = strp.tile([1, TPC], F32R, tag="ivr")
                nc.sync.dma_start(invrow[:], st_agg_v[1:2, c])
                stdrow = strp.tile([1, TPC], F32R, tag="std")
                nc.vector.reciprocal(stdrow[:].bitcast(F32),
                                     invrow[:].bitcast(F32))
                mub = strp.tile([1, TPC], BF16, tag="mub")
                nc.vector.tensor_copy(mub[:], murow[:])
                stdb = strp.tile([1, TPC], BF16, tag="stdb")
                nc.vector.tensor_copy(stdb[:], stdrow[:])
                invsb = strp.tile([128, TPC], F32, tag="invsb")
                qk_ps = []
                for m in range(2):
                    ps = psA.tile([128, TPC], F32, tag="ps")
                    chi = slice(128 * m, 128 * m + 128)
                    clo = slice(384 + 128 * m, 384 + 128 * m + 128)
                    for k in range(PO // 2):
                        ksl = slice(2 * k, 2 * k + 2)
                        nc.tensor.matmul(ps[:], wqkv_sb[:, ksl, chi],
                                         xc[:, ksl, :], start=(k == 0),
                                         stop=False, perf_mode=DR)
                        nc.tensor.matmul(ps[:], wqkv_sb[:, ksl, clo],
                                         xc[:, ksl, :], start=False,
                                         stop=False, perf_mode=DR)
                    qk_ps.append(ps)
                vt_ps = []
                for kt in range(4):   # V in keys-major layout
                    vsl = slice(128 * kt, 128 * kt + 128)
                    psv = psVt.tile([128, 128], F32, tag="psv")
                    for k in range(PO // 2):
                        ksl = slice(2 * k, 2 * k + 2)
                        nc.tensor.matmul(
                            psv[:], xc[:, ksl, vsl],
                            wqkv_sb[:, ksl, 256:384], start=(k == 0),
                            stop=False, perf_mode=DR)
                        nc.tensor.matmul(
                            psv[:], xc[:, ksl, vsl],
                            wqkv_sb[:, ksl, 640:768], start=False,
                            stop=False, perf_mode=DR)
                    vt_ps.append(psv)
                # stats-dependent tail: inv broadcast + rank-1 corrections
                psb = psbp.tile([128, TPC], F32, tag="psb")
                nc.tensor.matmul(psb[:], ones_row_r[:], invrow[:],
                                 start=True, stop=True)
                nc.scalar.activation(invsb[:], psb[:], AF.Copy)
                for m in range(2):
                    ps = qk_ps[m]
                    # ps holds 16*G: corrections are scaled by 16 to match
                    nc.tensor.matmul(ps[:], sqkv[:, 128 * m:128 * m + 128],
                                     murow[:], start=False, stop=False)
                    nc.tensor.matmul(ps[:], bqkr[:, 128 * m:128 * m + 128],
                                     stdrow[:], start=False, stop=True)
                nc.vector.scalar_tensor_tensor(
                    out=Qh[:, tok], in0=qk_ps[0][:], scalar=1.0 / 16.0,
                    in1=invsb[:], op0=ALU.mult, op1=ALU.mult)
                nc.vector.tensor_scalar_mul(Kh[:, tok], qk_ps[1][:],
                                            1.0 / 16.0)
                for kt in range(4):
                    g = 4 * c + kt
                    psv = vt_ps[kt]
                    ksl = slice(128 * kt, 128 * kt + 128)
                    nc.tensor.matmul(psv[:], mub[:, ksl], sv_row[:],
                                     start=False, stop=False)
                    nc.tensor.matmul(psv[:], stdb[:, ksl], bv_row[:],
                                     start=False, stop=True)
                    for h in range(HPC):
                        nc.vector.tensor_scalar(
                            out=Vt[:, h, g, 0:64],
                            in0=psv[:, 64 * h:64 * h + 64],
                            scalar1=1.0 / 16.0,
                            scalar2=invc[:, g:g + 1].bitcast(F32),
                            op0=ALU.mult, op1=ALU.mult)

            # residual x + Wo weights: needed from phase 4 on
            nc.sync.dma_start(xl[:],
                              xf_in.ap().rearrange("(po p) t -> p po t", p=128))
            nc.sync.dma_start(
                wo_sb[:], wo_in.ap().rearrange("(po p) n -> p po n", p=128))

        # ============ Phase 3: causal attention ============
        with tc.tile_pool(name="ptp", bufs=4) as ptp, \
             tc.tile_pool(name="rcd", bufs=2) as rcdp, \
             tc.tile_pool(name="avp", bufs=2) as avp, \
             tc.tile_pool(name="psS", bufs=2, space="PSUM") as psS, \
             tc.tile_pool(name="psV2", bufs=2, space="PSUM") as psV2:
            pending = []
            a2a_pairs = [(a2ai0, a2ao0), (a2ai1, a2ao1)]
            for h in range(HPC):
                hsl = slice(64 * h, 64 * h + 64)
                for b in range(B):
                    for qh in range(2):
                        n_i = 8 if qh == 0 else 16
                        ps_av = psV2.tile([65, 1024], F32, tag="psav")
                        for i in range(n_i):
                            if i == 2 and pending:
                                pending.pop()()
                            koff = 2048 * b + 128 * i
                            qlo = max(1024 * qh, 128 * i)
                            free = 1024 * (qh + 1) - qlo
                            off = qlo - 1024 * qh
                            qabs = 2048 * b + qlo
                            # matmul outputs must stay inside one psum bank
                            # (512 f32): emit per-bank-half pieces, aligned
                            # to each psum tile's own banks.
                            pieces = []   # ps_av-aligned (offset off..1024)
                            lo = off
                            while lo < 1024:
                                hi = min(1024, (lo // 512 + 1) * 512)
                                pieces.append((lo, hi - lo))
                                lo = hi
                            spieces = []  # pss-aligned (offset 0..free)
                            lo = 0
                            while lo < free:
                                hi = min(free, (lo // 512 + 1) * 512)
                                spieces.append((lo, hi - lo))
                                lo = hi
                            pss = psS.tile([128, 1024], F32, tag="pss")
                            for (plo, pfree) in spieces:
                                qa = qabs + plo
                                nc.tensor.matmul(
                                    pss[:, plo:plo + pfree],
                                    Kh[hsl, koff:koff + 128],
                                    Qh[hsl, qa:qa + pfree],
                                    start=True, stop=True)
                            pt = ptp.tile([128, 1024], BF16, tag="pt")
                            nc.scalar.activation(
                                pt[:, 0:free], pss[:, 0:free], AF.Exp,
                                scale=invc8[:, 16 * b + i:16 * b + i + 1]
                                .bitcast(F32))
                            if 128 * i >= 1024 * qh:   # diagonal tile
                                if h == 0:   # Pool is free before A2A#0
                                    nc.gpsimd.affine_select(
                                        out=pt[:, 0:128], in_=pt[:, 0:128],
                                        compare_op=ALU.is_ge, fill=0.0,
                                        base=0, pattern=[[1, 128]],
                                        channel_multiplier=-1)
                                else:        # Pool busy with A2A#0
                                    nc.vector.tensor_mul(pt[:, 0:128],
                                                         pt[:, 0:128],
                                                         tri[:])
                            for (plo, pfree) in pieces:
                                hb = plo // 512
                                last_i = 8 * qh + 3 if hb == 0 else n_i - 1
                                nc.tensor.matmul(
                                    ps_av[:, plo:plo + pfree],
                                    Vt[:, h, 16 * b + i, :],
                                    pt[:, plo - off:plo - off + pfree],
                                    start=(i == 0), stop=(i == last_i))
                        # epilogue: copy to SBUF + recip on DVE; the PE
                        # broadcast is deferred into the next group's loop
                        # so it doesn't head-of-line block the next scores.
                        avs = avp.tile([65, 1024], F32, tag="avs")
                        nc.vector.tensor_copy(avs[:], ps_av[:])
                        recd = rcdp.tile([1, 1024], F32R, tag="recd")
                        nc.vector.reciprocal(recd[:].bitcast(F32),
                                             avs[64:65, :])

                        def epilogue(avs=avs, recd=recd, b=b, qh=qh,
                                     h=h):
                            ps_bc = psS.tile([128, 1024], F32, tag="pss")
                            for half in range(2):
                                nc.tensor.matmul(
                                    ps_bc[0:64, 512 * half:512 * half + 512],
                                    ones_row_r[:, 0:64],
                                    recd[:, 512 * half:512 * half + 512],
                                    start=True, stop=True)
                            avn = avp.tile([64, 1024], FP8, tag="avn")
                            nc.vector.tensor_mul(avn[:], avs[0:64, :],
                                                 ps_bc[0:64, :])
                            g0 = 4 * b + 2 * qh
                            a2aih = a2a_pairs[h][0]
                            nc.sync.dma_start(a2aih[g0, :, :], avn[:, 0:TPC])
                            nc.sync.dma_start(a2aih[g0 + 1, :, :],
                                              avn[:, TPC:2 * TPC])
                        pending.append(epilogue)

                if b == B - 1:      # this head-half is complete
                    while pending:
                        pending.pop()()
                    if h == 0:      # launch early; h=1's goes after the
                        nc.gpsimd.collective_compute(   # pool scope closes
                            "AllToAll", ALU.bypass, replica_groups=RG,
                            ins=[a2ai0[:].opt()], outs=[a2ao0[:].opt()])

        qkvp_cm.__exit__(None, None, None)   # free Qh/Kh/Vt

        # ==== Phases 4-6 in one pool scope: everything here is emitted
        # before the second AllToAll so pool-open barriers don't serialize
        # on it; Wo's first half + weight prefetches overlap the collective.
        with tc.tile_pool(name="avtp", bufs=1) as avtp, \
             tc.tile_pool(name="w1p", bufs=2) as w1p, \
             tc.tile_pool(name="w2p", bufs=2) as w2p, \
             tc.tile_pool(name="tmp2", bufs=2) as tmp2p, \
             tc.tile_pool(name="sgp", bufs=2) as sgp, \
             tc.tile_pool(name="outp", bufs=2) as outp, \
             tc.tile_pool(name="psA2", bufs=8, space="PSUM") as psA2:
            psB = psA2
            w1_sb = []
            w2_sb = []
            for g in range(4):   # prefetch during the A2A / Wo phase
                w1t = w1p.tile([128, PO, 2048], FP8, tag="w1")
                nc.sync.dma_start(
                    w1t[:, :, 0:1024], wf1_in.ap()[:, 1024 * g:1024 * (g + 1)]
                    .rearrange("(po p) n -> p po n", p=128))
                nc.sync.dma_start(
                    w1t[:, :, 1024:2048],
                    wf1_in.ap()[:, DFF + 1024 * g:DFF + 1024 * (g + 1)]
                    .rearrange("(po p) n -> p po n", p=128))
                w1_sb.append(w1t)
            for g in range(4):
                w2t = w2p.tile([128, M1, 512], FP8, tag="w2")
                nc.sync.dma_start(
                    w2t[:], wf2_in.ap()[:, 512 * g:512 * (g + 1)]
                    .rearrange("(ko p) n -> p ko n", p=128))
                w2_sb.append(w2t)

            AVt0 = avtp.tile([128, 4, TPC], FP8, tag="avt0")
            nc.scalar.dma_start(
                AVt0[:],
                a2ao0[:].rearrange("(po s2) p t -> (s2 p) po t", s2=2))
            wo_ps = []
            for m in range(PO):   # h0 half: overlaps the second AllToAll
                ps = psA2.tile([128, TPC], F32, tag="ps")
                chi = slice(128 * m, 128 * m + 128)
                clo = slice(D + 128 * m, D + 128 * m + 128)
                for k in range(2):
                    ksl = slice(2 * k, 2 * k + 2)
                    nc.tensor.matmul(ps[:], wo_sb[:, ksl, chi],
                                     AVt0[:, ksl, :], start=(k == 0),
                                     stop=False, perf_mode=DR)
                    nc.tensor.matmul(ps[:], wo_sb[:, ksl, clo],
                                     AVt0[:, ksl, :], start=False,
                                     stop=False, perf_mode=DR)
                wo_ps.append(ps)

            nc.gpsimd.collective_compute(
                "AllToAll", ALU.bypass, replica_groups=RG,
                ins=[a2ai1[:].opt()], outs=[a2ao1[:].opt()])
            AVt1 = avtp.tile([128, 4, TPC], FP8, tag="avt1")
            nc.scalar.dma_start(
                AVt1[:],
                a2ao1[:].rearrange("(po s2) p t -> (s2 p) po t", s2=2))

            for m in range(PO):   # h1 half + epilogue
                ps = wo_ps[m]
                chi = slice(128 * m, 128 * m + 128)
                clo = slice(D + 128 * m, D + 128 * m + 128)
                for k in range(2):
                    ksl = slice(2 * k, 2 * k + 2)
                    nc.tensor.matmul(ps[:], wo_sb[:, 4 + 2 * k:6 + 2 * k, chi],
                                     AVt1[:, ksl, :], start=False,
                                     stop=False, perf_mode=DR)
                    nc.tensor.matmul(ps[:], wo_sb[:, 4 + 2 * k:6 + 2 * k, clo],
                                     AVt1[:, ksl, :], start=False,
                                     stop=False, perf_mode=DR)
                nc.tensor.matmul(ps[:], bo_r[:, 128 * m:128 * m + 128],
                                 ones_tok[:], start=False, stop=True)
                nc.vector.scalar_tensor_tensor(
                    out=X2[:, m, :].bitcast(F32), in0=ps[:],
                    scalar=1.0 / 16.0, in1=xl[:, m, :],
                    op0=ALU.mult, op1=ALU.add)

            ps_s2 = psB.tile([128, TPC], F32, tag="ps")
            for po in range(PO):
                nc.tensor.matmul(ps_s2[0:1, :], ones_col_r[:],
                                 X2[:, po, :],
                                 start=(po == 0), stop=(po == PO - 1))
            ps_q2 = psB.tile([128, TPC], F32, tag="ps")
            for po in range(PO):
                sq = tmp2p.tile([128, TPC], F32R, tag="sq2")
                nc.scalar.activation(sq[:].bitcast(F32),
                                     X2[:, po, :].bitcast(F32), AF.Square)
                nc.tensor.matmul(ps_q2[0:1, :], ones_col_r[:], sq[:],
                                 start=(po == 0), stop=(po == PO - 1))
            nmu2 = rows.tile([1, TPC], F32R, tag="nmu2")
            nc.scalar.activation(nmu2[:].bitcast(F32), ps_s2[0:1, :],
                                 AF.Copy, scale=-1.0 / D)
            ex2b = rows.tile([1, TPC], F32, tag="ex2b")
            nc.scalar.activation(ex2b[:], ps_q2[0:1, :], AF.Copy,
                                 scale=1.0 / D)
            mu2b = rows.tile([1, TPC], F32, tag="mu2b")
            nc.vector.tensor_mul(mu2b[:], nmu2[:].bitcast(F32),
                                 nmu2[:].bitcast(F32))
            varb = rows.tile([1, TPC], F32, tag="varb")
            nc.vector.scalar_tensor_tensor(
                out=varb[:], in0=ex2b[:], scalar=EPS, in1=mu2b[:],
                op0=ALU.add, op1=ALU.subtract)
            recb = rows.tile([1, TPC], F32, tag="recb")
            nc.vector.reciprocal(recb[:], varb[:])
            inv2 = rows.tile([1, TPC], F32R, tag="inv2")
            nc.scalar.activation(inv2[:].bitcast(F32), recb[:], AF.Sqrt)
            ps_mu = psB.tile([128, TPC], F32, tag="ps")
            nc.tensor.matmul(ps_mu[:], ones_row_r[:], nmu2[:],
                             start=True, stop=True)
            ps_iv = psB.tile([128, TPC], F32, tag="ps")
            nc.tensor.matmul(ps_iv[:], ones_row_r[:], inv2[:],
                             start=True, stop=True)
            for po in range(PO):
                t0 = tmp2p.tile([128, TPC], F32, tag="t0")
                nc.vector.tensor_add(t0[:], X2[:, po, :].bitcast(F32),
                                     ps_mu[:])
                nc.vector.tensor_mul(xh2[:, po, :], t0[:], ps_iv[:])

            for m in range(M1):
                ps = psB.tile([128, TPC], F32, tag="ps")
                w1t = w1_sb[m // 8]
                chi = slice(128 * (m % 8), 128 * (m % 8) + 128)
                clo = slice(1024 + 128 * (m % 8), 1024 + 128 * (m % 8) + 128)
                for k in range(PO // 2):
                    ksl = slice(2 * k, 2 * k + 2)
                    nc.tensor.matmul(ps[:], w1t[:, ksl, chi],
                                     xh2[:, ksl, :], start=(k == 0),
                                     stop=False, perf_mode=DR)
                    nc.tensor.matmul(ps[:], w1t[:, ksl, clo],
                                     xh2[:, ksl, :], start=False,
                                     stop=(k == PO // 2 - 1), perf_mode=DR)
                sg = sgp.tile([128, TPC], BF16, tag="sg")
                nc.scalar.activation(sg[:], ps[:], AF.Sigmoid,
                                     scale=SIGC / 16.0,
                                     bias=bf1s[:, m:m + 1])
                nc.vector.scalar_tensor_tensor(
                    out=Amat[:, m, :], in0=ps[:],
                    scalar=bf1[:, m:m + 1], in1=sg[:],
                    op0=ALU.add, op1=ALU.mult)
            out_view = out_t.ap().rearrange("(po p) t -> p po t", p=128)
            for m in range(PO):
                ps = psB.tile([128, TPC], F32, tag="ps")
                w2t = w2_sb[m // 2]
                chi = slice(128 * (m % 2), 128 * (m % 2) + 128)
                clo = slice(256 + 128 * (m % 2), 256 + 128 * (m % 2) + 128)
                for k in range(M1 // 2):
                    ksl = slice(2 * k, 2 * k + 2)
                    nc.tensor.matmul(ps[:], w2t[:, ksl, chi],
                                     Amat[:, ksl, :], start=(k == 0),
                                     stop=False, perf_mode=DR)
                    nc.tensor.matmul(ps[:], w2t[:, ksl, clo],
                                     Amat[:, ksl, :], start=False,
                                     stop=False, perf_mode=DR)
                nc.tensor.matmul(ps[:], bf2_r[:, 128 * m:128 * m + 128],
                                 ones_tok[:], start=False, stop=True)
                om = outp.tile([128, TPC], F32, tag="om")
                nc.vector.scalar_tensor_tensor(
                    out=om[:], in0=ps[:], scalar=1.0 / 256.0,
                    in1=X2[:, m, :].bitcast(F32),
                    op0=ALU.mult, op1=ALU.add)
                nc.sync.dma_start(out_view[:, m, :], om[:])
        xlp_cm.__exit__(None, None, None)    # free xl/wo_sb

    nc.compile()
    return nc


def _get_nc():
    key = ("nc", GELU_NATIVE)
    if key not in _CACHE:
        _CACHE[key] = _build()
    return _CACHE[key]


def _make_in_maps(inputs):
    x = np.asarray(inputs["x"], np.float32).reshape(BT, D)
    ln1w = np.asarray(inputs["ln1_w"], np.float32)
    ln1b = np.asarray(inputs["ln1_b"], np.float32)
    ln2w = np.asarray(inputs["ln2_w"], np.float32)
    ln2b = np.asarray(inputs["ln2_b"], np.float32)
    W_qkv0 = np.asarray(inputs["W_qkv"], np.float32)
    W_qkv = W_qkv0 * ln1w[:, None]
    b_qkv = np.asarray(inputs["b_qkv"], np.float32) + ln1b @ W_qkv0
    W_o = np.asarray(inputs["W_o"], np.float32)
    b_o = np.asarray(inputs["b_o"], np.float32)
    W_ff10 = np.asarray(inputs["W_ff1"], np.float32)
    W_ff1 = W_ff10 * ln2w[:, None]
    b_ff1 = np.asarray(inputs["b_ff1"], np.float32) + ln2b @ W_ff10
    W_ff2 = np.asarray(inputs["W_ff2"], np.float32)
    b_ff2 = np.asarray(inputs["b_ff2"], np.float32)

    def pcol(v):  # [D'] -> [128, D'/128] per-partition column layout
        return np.ascontiguousarray(v.reshape(-1, 128).T)

    xT = np.ascontiguousarray(x.T)                      # [D, BT] f32

    def hilo16(w):   # 16*w as fp8 hi + fp8 residual, concatenated wide
        base = (16.0 * w).astype(np.float32)
        hi = base.astype(NPFP8)
        lo = (base - hi.astype(np.float32)).astype(NPFP8)
        return np.ascontiguousarray(np.concatenate([hi, lo], axis=1))

    def hilo16_blocked(w, blk):   # [hi0|lo0|hi1|lo1|...] per blk columns
        base = (16.0 * w).astype(np.float32)
        hi = base.astype(NPFP8)
        lo = (base - hi.astype(np.float32)).astype(NPFP8)
        parts = []
        for g in range(w.shape[1] // blk):
            parts.append(hi[:, blk * g:blk * (g + 1)])
            parts.append(lo[:, blk * g:blk * (g + 1)])
        return np.ascontiguousarray(np.concatenate(parts, axis=1))

    common = {
        "xb": xT.astype(NPFP8),
        "wo": hilo16(W_o[WO_PERM]),
        "bo": np.ascontiguousarray(16.0 * b_o).reshape(1, D).astype(NPBF16),
        "wf1": hilo16(W_ff1),
        "bf1": pcol(16.0 * b_ff1),
        "bf1s": pcol(SIGC * b_ff1).astype(np.float32),
        "wf2": hilo16_blocked(W_ff2, 256),
        "bf2": np.ascontiguousarray(
            256.0 * b_ff2).reshape(1, D).astype(NPBF16),
    }
    in_maps = []
    for r in range(NC_N):
        hc = 128 * r          # first column of this core's Q/K/V head block
        m = dict(common)
        m["xs"] = np.ascontiguousarray(
            xT[:, TPC * r:TPC * (r + 1)]).astype(NPBF16)
        m["xf"] = np.ascontiguousarray(xT[:, TPC * r:TPC * (r + 1)])
        wq = W_qkv[:, hc:hc + 128]
        wk = W_qkv[:, D + hc:D + hc + 128]
        wv = W_qkv[:, 2 * D + hc:2 * D + hc + 128]
        m["wqkv"] = hilo16(np.concatenate([wq, wk, wv], axis=1))
        wdq = (m["wqkv"][:, 0:384].astype(np.float32)
               + m["wqkv"][:, 384:768].astype(np.float32))   # = 16*W'
        m["sqkv"] = np.ascontiguousarray(
            wdq[:, 0:256].sum(0).reshape(1, 256)).astype(np.float32)
        m["svb"] = np.ascontiguousarray(np.stack(
            [wdq[:, 256:384].sum(0),
             16.0 * b_qkv[2 * D + hc:2 * D + hc + 128]])).astype(NPBF16)
        m["bqkr"] = np.ascontiguousarray(16.0 * np.concatenate(
            [b_qkv[hc:hc + 128], b_qkv[D + hc:D + hc + 128]])
            .reshape(1, 256)).astype(np.float32)
        in_maps.append(m)
    return in_maps


def _run_sim(nc, in_maps):
    """Instruction-level simulator fallback executor (same program)."""
    from concourse.bass_interp import MultiCoreSim
    sim = MultiCoreSim(nc, num_cores=NC_N, require_finite=False)
    for i in range(NC_N):
        for k, v in in_maps[i].items():
            sim.cores[i].tensor(k)[:] = np.asarray(v)
    sim.simulate(check_with_hw=False)
    return [np.array(sim.cores[i].tensor("outt")) for i in range(NC_N)]


def _run(inputs, trace=False, trace_cores=None):
    nc = _get_nc()
    in_maps = _make_in_maps(inputs)
    res = None
    try:
        res = bass_utils.run_bass_kernel_spmd(
            nc, in_maps, core_ids=list(range(NC_N)), trace=trace,
            trace_cores=trace_cores)
        outs = [res.results[r]["outt"] for r in range(NC_N)]
    except Exception:
        outs = _run_sim(nc, in_maps)
    full = np.concatenate([np.asarray(o, np.float32).T for o in outs], axis=0)
    return full.reshape(B, T, D).astype(np.float32), res


def kernel(**inputs):
    out, _ = _run(inputs, trace=False)
    return out


# revision 63
# speedup vs baseline: 1.0180x; 1.0150x over previous
"""Trainium2 Bass kernel for a dense transformer block (nn_Block_88338887344891).

Distribution over 8 NeuronCores (single SPMD NEFF, 2 collectives):
  - LN1 stats computed per-core on its own 512 tokens, AllGathered (48KB,
    ~16us, overlapped with the QKV matmuls on raw x).
  - LayerNorm is folded into the matmuls: QKV = inv * (W'^T x + s (-mu)) + b'
    where W' = diag(ln_w) W (host-folded), s = colsum(W'), so the heavy
    matmuls start before the stats arrive; ln_b is folded into b'.
  - QKV + causal attention head-sharded (2 heads/core over all 4096 tokens);
    scores/exp/AV trimmed to the causal region; V is produced directly in
    keys-major layout (x as the stationary operand) so no transposes.
  - attention output AllToAll per head in bf16 (1MB, ~41us)
  - output projection + residual + LN2 + full MLP token-sharded (512 tok/core)
  - gelu(tanh approx) via t*sigmoid(1.702 t) (1 ACT + 1 fused DVE op)
All matmul moving operands are bf16 (1 cycle/row); PSUM accumulates f32.
"""
import numpy as np
from contextlib import ExitStack

try:  # persistent XLA cache so repeat runs skip the NEFF compile
    import jax
    jax.config.update("jax_compilation_cache_dir", "/tmp/jax_neff_cache")
    jax.config.update("jax_persistent_cache_min_compile_time_secs", 1.0)
except Exception:
    pass

import ml_dtypes
import concourse.bass as bass
import concourse.bacc as bacc
import concourse.tile as tile
import concourse.mybir as mybir
from concourse import bass_utils

AF = mybir.ActivationFunctionType
ALU = mybir.AluOpType
F32 = mybir.dt.float32
F32R = mybir.dt.float32r
BF16 = mybir.dt.bfloat16
FP8 = mybir.dt.float8e4
NPBF16 = ml_dtypes.bfloat16
NPFP8 = mybir.dt.np(mybir.dt.float8e4)
DR = mybir.MatmulPerfMode.DoubleRow

NC_N = 8          # cores
B, T, D, H = 2, 2048, 1024, 16
HD = D // H       # 64
DFF = 4 * D       # 4096
EPS = 1e-5
BT = B * T               # 4096 tokens
TPC = BT // NC_N         # 512 tokens per core
HPC = H // NC_N          # 2 heads per core
PO = D // 128            # 8 D-tiles
M1 = DFF // 128          # 32 ff1 out tiles
NKT = BT // 128          # 32 key tiles globally (16 per batch)
RG = [list(range(NC_N))]
SIGC = 1.702             # gelu sigmoid-form constant

GELU_NATIVE = False   # kept for test.py compat; kernel is sim/hw identical

# Wo row order after the per-head AllToAll halves: feature index
# n = half*512 + po*128 + s2*64 + d maps to old row 64*(2*(2*po+s2)+half)+d
WO_PERM = np.array([64 * (2 * (2 * po + s2) + half) + d
                    for half in range(2) for po in range(4)
                    for s2 in range(2) for d in range(64)])

_CACHE = {}


def _build():
    nc = bacc.Bacc("TRN2", target_bir_lowering=False, debug=False,
                   num_devices=NC_N)

    # ---- per-core external inputs ----
    xb_in = nc.dram_tensor("xb", [D, BT], FP8, kind="ExternalInput")
    xs_in = nc.dram_tensor("xs", [D, TPC], BF16, kind="ExternalInput")
    xf_in = nc.dram_tensor("xf", [D, TPC], F32, kind="ExternalInput")
    wqkv_in = nc.dram_tensor("wqkv", [D, 768], FP8, kind="ExternalInput")
    sqkv_in = nc.dram_tensor("sqkv", [1, 256], F32R, kind="ExternalInput")
    svb_in = nc.dram_tensor("svb", [2, 128], BF16, kind="ExternalInput")
    bqkr_in = nc.dram_tensor("bqkr", [1, 256], F32R, kind="ExternalInput")
    wo_in = nc.dram_tensor("wo", [D, 2 * D], FP8, kind="ExternalInput")
    bo_in = nc.dram_tensor("bo", [1, D], BF16, kind="ExternalInput")
    wf1_in = nc.dram_tensor("wf1", [D, 2 * DFF], FP8, kind="ExternalInput")
    bf1_in = nc.dram_tensor("bf1", [128, M1], F32, kind="ExternalInput")
    bf1s_in = nc.dram_tensor("bf1s", [128, M1], F32, kind="ExternalInput")
    wf2_in = nc.dram_tensor("wf2", [DFF, 2 * D], FP8, kind="ExternalInput")
    bf2_in = nc.dram_tensor("bf2", [1, D], BF16, kind="ExternalInput")
    out_t = nc.dram_tensor("outt", [D, TPC], F32, kind="ExternalOutput")

    with tile.TileContext(nc, pool_alloc_mode="queue") as tc, \
            ExitStack() as ctx:
        perm = ctx.enter_context(tc.tile_pool(name="perm", bufs=1))
        big = ctx.enter_context(tc.tile_pool(name="big", bufs=1))
        rows = ctx.enter_context(tc.tile_pool(name="rows", bufs=1))
        dram = ctx.enter_context(tc.tile_pool(name="dram", bufs=1, space="DRAM"))

        # ---- constants ----
        ones_col_b = perm.tile([128, 1], BF16)
        nc.vector.memset(ones_col_b[:], 1.0)
        ones_col_f = perm.tile([128, 1], F32)
        nc.vector.memset(ones_col_f[:], 1.0)
        ones_col_r = perm.tile([128, 1], F32R)
        nc.vector.tensor_copy(ones_col_r[:], ones_col_f[:])
        ones_row_f = perm.tile([1, 128], F32)
        nc.vector.memset(ones_row_f[:], 1.0)
        ones_row_r = perm.tile([1, 128], F32R)
        nc.vector.tensor_copy(ones_row_r[:], ones_row_f[:])
        ones_row_b = perm.tile([1, 128], BF16)
        nc.vector.tensor_copy(ones_row_b[:], ones_row_f[:])
        ones_tok = perm.tile([1, TPC], BF16)
        nc.vector.memset(ones_tok[:], 1.0)

        def load_const(t_in, shape, tag, dt=F32):
            t = perm.tile(shape, dt, tag=tag)
            nc.sync.dma_start(t[:], t_in.ap())
            return t

        sqkv = load_const(sqkv_in, [1, 256], "c_sqkv", F32R)
        sv_row = perm.tile([1, 128], BF16, tag="c_sv")
        nc.sync.dma_start(sv_row[:], svb_in.ap()[0:1])
        bv_row = perm.tile([1, 128], BF16, tag="c_bv")
        nc.sync.dma_start(bv_row[:], svb_in.ap()[1:2])
        bqkr = load_const(bqkr_in, [1, 256], "c_bqkr", F32R)
        bo_r = load_const(bo_in, [1, D], "c_bo", BF16)      # 16*b_o
        bf1 = load_const(bf1_in, [128, M1], "c_bf1")        # 16*b_ff1
        bf1s = load_const(bf1s_in, [128, M1], "c_bf1s")     # SIGC/16 bias
        bf2_r = load_const(bf2_in, [1, D], "c_bf2", BF16)   # 256*b_ff2

        # ---- persistent SBUF ----
        invc = big.tile([128, NKT], F32R, tag="invc")  # inv keys-major
        invc8 = big.tile([128, NKT], F32R, tag="invc8")  # 0.125*inv (exp)
        invc16 = big.tile([128, NKT], F32R, tag="invc16")  # inv/16 (V evac)
        X2 = big.tile([128, PO, TPC], F32R, tag="x2")
        xh2 = big.tile([128, PO, TPC], FP8, tag="xh2")
        Amat = big.tile([128, M1, TPC], FP8, tag="amat")

        # dram scratch
        st_out = dram.tile([2, TPC], F32R)
        st_agg = dram.tile([NC_N, 2, TPC], F32R, addr_space="Shared")
        a2ai0 = dram.tile([NC_N, 64, TPC], FP8)
        a2ao0 = dram.tile([NC_N, 64, TPC], FP8)
        a2ai1 = dram.tile([NC_N, 64, TPC], FP8)
        a2ao1 = dram.tile([NC_N, 64, TPC], FP8)

        xb_view = xb_in.ap().rearrange("(po p) (k t) -> k p po t",
                                       p=128, t=TPC)

        xlp_cm = tc.tile_pool(name="xlp", bufs=1)
        xlp = xlp_cm.__enter__()
        xl = xlp.tile([128, PO, TPC], F32, tag="xl")   # my x (residual)
        wo_sb = xlp.tile([128, PO, 2 * D], FP8, tag="wo")
        qkvp_cm = tc.tile_pool(name="qkvp", bufs=1)
        qkvp = qkvp_cm.__enter__()
        Qh = qkvp.tile([128, BT], BF16, tag="qh")      # 2 heads stacked
        Kh = qkvp.tile([128, BT], BF16, tag="kh")
        Vt = qkvp.tile([128, HPC, NKT, 65], BF16, tag="vt")  # keys-major V
        nc.gpsimd.memset(Vt[:, :, :, 64:65], 1.0)   # softmax denominator row
        tri = perm.tile([128, 128], BF16, tag="tri")  # tri[p,q]=1 iff q>=p
        nc.gpsimd.memset(tri[:], 1.0)
        nc.gpsimd.affine_select(
            out=tri[:], in_=tri[:], compare_op=ALU.is_ge, fill=0.0,
            base=0, pattern=[[1, 128]], channel_multiplier=-1)

        # ============ Phase 1: local LN1 stats + AllGather ============
        with tc.tile_pool(name="wq", bufs=1) as wq, \
             tc.tile_pool(name="xsp", bufs=1) as xsp, \
             tc.tile_pool(name="xcp", bufs=2) as xcp, \
             tc.tile_pool(name="sqp", bufs=2) as sqp, \
             tc.tile_pool(name="strp", bufs=2) as strp, \
             tc.tile_pool(name="psA", bufs=4, space="PSUM") as psA, \
             tc.tile_pool(name="psBb", bufs=2, space="PSUM") as psbp, \
             tc.tile_pool(name="psV", bufs=2, space="PSUM") as psVt:
            xst = xsp.tile([128, PO, TPC], BF16)
            nc.sync.dma_start(
                xst[:], xs_in.ap().rearrange("(po p) t -> p po t", p=128))
            wqkv_sb = wq.tile([128, PO, 768], FP8)
            nc.sync.dma_start(
                wqkv_sb[:], wqkv_in.ap().rearrange("(po p) m -> p po m", p=128))
            xc0 = xcp.tile([128, PO, TPC], FP8, tag="xc")
            nc.sync.dma_start(xc0[:], xb_view[0])

            # stats on my 512 tokens
            stmu_l = rows.tile([1, TPC], F32R, tag="stmu_l")
            stinv_l = rows.tile([1, TPC], F32R, tag="stinv_l")
            ps_s = psA.tile([128, TPC], F32, tag="ps")
            for po in range(PO):
                nc.tensor.matmul(ps_s[0:1, :], ones_col_b[:], xst[:, po, :],
                                 start=(po == 0), stop=(po == PO - 1))
            ps_q = psA.tile([128, TPC], F32, tag="ps")
            for po in range(PO):
                sq = sqp.tile([128, TPC], BF16, tag="sq")
                nc.vector.tensor_mul(sq[:], xst[:, po, :], xst[:, po, :])
                nc.tensor.matmul(ps_q[0:1, :], ones_col_b[:], sq[:],
                                 start=(po == 0), stop=(po == PO - 1))
            nc.scalar.activation(stmu_l[:].bitcast(F32), ps_s[0:1, :],
                                 AF.Copy, scale=-1.0 / D)     # -mu
            ex2 = rows.tile([1, TPC], F32, tag="ex2")
            nc.scalar.activation(ex2[:], ps_q[0:1, :], AF.Copy, scale=1.0 / D)
            mu2 = rows.tile([1, TPC], F32, tag="mu2")
            nc.vector.tensor_mul(mu2[:], stmu_l[:].bitcast(F32),
                                 stmu_l[:].bitcast(F32))
            var = rows.tile([1, TPC], F32, tag="var")
            nc.vector.scalar_tensor_tensor(
                out=var[:], in0=ex2[:], scalar=EPS, in1=mu2[:],
                op0=ALU.add, op1=ALU.subtract)
            rec = rows.tile([1, TPC], F32, tag="rec")
            nc.vector.reciprocal(rec[:], var[:])
            nc.scalar.activation(stinv_l[:].bitcast(F32), rec[:],
                                 AF.Sqrt)                      # inv = 1/std
            nc.sync.dma_start(st_out[0:1, :], stmu_l[:])
            nc.sync.dma_start(st_out[1:2, :], stinv_l[:])
            nc.gpsimd.collective_compute(
                "AllGather", ALU.bypass, replica_groups=RG,
                ins=[st_out[:].opt()], outs=[st_agg[:].opt()])
            # gathered stats (ACT dma queue: don't block SP chunk stream)
            st_agg_v = st_agg[:].rearrange("s r t -> r s t")
            # inv keys-major + 0.125*inv for the exp scale (K's LN fold)
            st_agg_c = st_agg[:].rearrange("s r (c p) -> s r p c", p=128)
            for s in range(NC_N):
                nc.gpsimd.dma_start(invc[:, 4 * s:4 * s + 4], st_agg_c[s, 1])
            nc.vector.tensor_scalar_mul(invc8[:].bitcast(F32),
                                        invc[:].bitcast(F32), 0.125)
            nc.vector.tensor_scalar_mul(invc16[:].bitcast(F32),
                                        invc[:].bitcast(F32), 1.0 / 16.0)

            # ===== Phase 2: QKV, one fused pass =====
            # raw-G DoubleRow matmuls carry no stats dependency; the rank-1
            # LN corrections (s (x) -mu, b (x) std) append to each psum
            # group once the 48KB stats AllGather lands.
            for c in range(NC_N):
                if c == 0:
                    xc = xc0
                else:
                    xc = xcp.tile([128, PO, TPC], FP8, tag="xc")
                    nc.sync.dma_start(xc[:], xb_view[c])
                tok = slice(TPC * c, TPC * (c + 1))
                murow = strp.tile([1, TPC], F32R, tag="mur")
                nc.gpsimd.dma_start(murow[:], st_agg_v[0:1, c])
                invrow = strp.tile([1, TPC], F32R, tag="ivr")
                nc.sync.dma_start(invrow[:], st_agg_v[1:2, c])
                stdrow = strp.tile([1, TPC], F32R, tag="std")
                nc.vector.reciprocal(stdrow[:].bitcast(F32),
                                     invrow[:].bitcast(F32))
                mub = strp.tile([1, TPC], BF16, tag="mub")
                nc.vector.tensor_copy(mub[:], murow[:])
                stdb = strp.tile([1, TPC], BF16, tag="stdb")
                nc.vector.tensor_copy(stdb[:], stdrow[:])
                invsb = strp.tile([128, TPC], F32, tag="invsb")
                qk_ps = []
                for m in range(2):
                    ps = psA.tile([128, TPC], F32, tag="ps")
                    chi = slice(128 * m, 128 * m + 128)
                    clo = slice(384 + 128 * m, 384 + 128 * m + 128)
                    for k in range(PO // 2):
                        ksl = slice(2 * k, 2 * k + 2)
                        nc.tensor.matmul(ps[:], wqkv_sb[:, ksl, chi],
                                         xc[:, ksl, :], start=(k == 0),
                                         stop=False, perf_mode=DR)
                        nc.tensor.matmul(ps[:], wqkv_sb[:, ksl, clo],
                                         xc[:, ksl, :], start=False,
                                         stop=False, perf_mode=DR)
                    qk_ps.append(ps)
                vt_ps = []
                for kt in range(4):   # V in keys-major layout
                    vsl = slice(128 * kt, 128 * kt + 128)
                    psv = psVt.tile([128, 128], F32, tag="psv")
                    for k in range(PO // 2):
                        ksl = slice(2 * k, 2 * k + 2)
                        nc.tensor.matmul(
                            psv[:], xc[:, ksl, vsl],
                            wqkv_sb[:, ksl, 256:384], start=(k == 0),
                            stop=False, perf_mode=DR)
                        nc.tensor.matmul(
                            psv[:], xc[:, ksl, vsl],
                            wqkv_sb[:, ksl, 640:768], start=False,
                            stop=False, perf_mode=DR)
                    vt_ps.append(psv)
                # stats-dependent tail: inv broadcast + rank-1 corrections
                psb = psbp.tile([128, TPC], F32, tag="psb")
                nc.tensor.matmul(psb[:], ones_row_r[:], invrow[:],
                                 start=True, stop=True)
                nc.scalar.activation(invsb[:], psb[:], AF.Copy)
                for m in range(2):
                    ps = qk_ps[m]
                    # ps holds 16*G: corrections are scaled by 16 to match
                    nc.tensor.matmul(ps[:], sqkv[:, 128 * m:128 * m + 128],
                                     murow[:], start=False, stop=False)
                    nc.tensor.matmul(ps[:], bqkr[:, 128 * m:128 * m + 128],
                                     stdrow[:], start=False, stop=True)
                nc.vector.scalar_tensor_tensor(
                    out=Qh[:, tok], in0=qk_ps[0][:], scalar=1.0 / 16.0,
                    in1=invsb[:], op0=ALU.mult, op1=ALU.mult)
                nc.vector.tensor_scalar_mul(Kh[:, tok], qk_ps[1][:],
                                            1.0 / 16.0)
                for kt in range(4):
                    g = 4 * c + kt
                    psv = vt_ps[kt]
                    ksl = slice(128 * kt, 128 * kt + 128)
                    nc.tensor.matmul(psv[:], mub[:, ksl], sv_row[:],
                                     start=False, stop=False)
                    nc.tensor.matmul(psv[:], stdb[:, ksl], bv_row[:],
                                     start=False, stop=True)
                    for h in range(HPC):
                        nc.vector.tensor_scalar(
                            out=Vt[:, h, g, 0:64],
                            in0=psv[:, 64 * h:64 * h + 64],
                            scalar1=1.0 / 16.0,
                            scalar2=invc[:, g:g + 1].bitcast(F32),
                            op0=ALU.mult, op1=ALU.mult)

            # residual x + Wo weights: needed from phase 4 on
            nc.sync.dma_start(xl[:],
                              xf_in.ap().rearrange("(po p) t -> p po t", p=128))
            nc.sync.dma_start(
                wo_sb[:], wo_in.ap().rearrange("(po p) n -> p po n", p=128))

        # ============ Phase 3: causal attention ============
        with tc.tile_pool(name="ptp", bufs=4) as ptp, \
             tc.tile_pool(name="rcd", bufs=2) as rcdp, \
             tc.tile_pool(name="avp", bufs=2) as avp, \
             tc.tile_pool(name="psS", bufs=2, space="PSUM") as psS, \
             tc.tile_pool(name="psV2", bufs=2, space="PSUM") as psV2:
            pending = []
            a2a_pairs = [(a2ai0, a2ao0), (a2ai1, a2ao1)]
            for h in range(HPC):
                hsl = slice(64 * h, 64 * h + 64)
                for b in range(B):
                    for qh in range(2):
                        n_i = 8 if qh == 0 else 16
                        ps_av = psV2.tile([65, 1024], F32, tag="psav")
                        for i in range(n_i):
                            if i == 2 and pending:
                                pending.pop()()
                            koff = 2048 * b + 128 * i
                            qlo = max(1024 * qh, 128 * i)
                            free = 1024 * (qh + 1) - qlo
                            off = qlo - 1024 * qh
                            qabs = 2048 * b + qlo
                            # matmul outputs must stay inside one psum bank
                            # (512 f32): emit per-bank-half pieces, aligned
                            # to each psum tile's own banks.
                            pieces = []   # ps_av-aligned (offset off..1024)
                            lo = off
                            while lo < 1024:
                                hi = min(1024, (lo // 512 + 1) * 512)
                                pieces.append((lo, hi - lo))
                                lo = hi
                            spieces = []  # pss-aligned (offset 0..free)
                            lo = 0
                            while lo < free:
                                hi = min(free, (lo // 512 + 1) * 512)
                                spieces.append((lo, hi - lo))
                                lo = hi
                            pss = psS.tile([128, 1024], F32, tag="pss")
                            for (plo, pfree) in spieces:
                                qa = qabs + plo
                                nc.tensor.matmul(
                                    pss[:, plo:plo + pfree],
                                    Kh[hsl, koff:koff + 128],
                                    Qh[hsl, qa:qa + pfree],
                                    start=True, stop=True)
                            pt = ptp.tile([128, 1024], BF16, tag="pt")
                            nc.scalar.activation(
                                pt[:, 0:free], pss[:, 0:free], AF.Exp,
                                scale=invc8[:, 16 * b + i:16 * b + i + 1]
                                .bitcast(F32))
                            if 128 * i >= 1024 * qh:   # diagonal tile
                                if h == 0:   # Pool is free before A2A#0
                                    nc.gpsimd.affine_select(
                                        out=pt[:, 0:128], in_=pt[:, 0:128],
                                        compare_op=ALU.is_ge, fill=0.0,
                                        base=0, pattern=[[1, 128]],
                                        channel_multiplier=-1)
                                else:        # Pool busy with A2A#0
                                    nc.vector.tensor_mul(pt[:, 0:128],
                                                         pt[:, 0:128],
                                                         tri[:])
                            for (plo, pfree) in pieces:
                                hb = plo // 512
                                last_i = 8 * qh + 3 if hb == 0 else n_i - 1
                                nc.tensor.matmul(
                                    ps_av[:, plo:plo + pfree],
                                    Vt[:, h, 16 * b + i, :],
                                    pt[:, plo - off:plo - off + pfree],
                                    start=(i == 0), stop=(i == last_i))
                        # epilogue: copy to SBUF + recip on DVE; the PE
                        # broadcast is deferred into the next group's loop
                        # so it doesn't head-of-line block the next scores.
                        recd = rcdp.tile([1, 1024], F32R, tag="recd")
                        nc.vector.reciprocal(recd[:].bitcast(F32),
                                             ps_av[64:65, :])
                        avs = avp.tile([65, 1024], F32, tag="avs")
                        nc.vector.tensor_copy(avs[0:64, :], ps_av[0:64, :])

                        def epilogue(avs=avs, recd=recd, b=b, qh=qh,
                                     h=h):
                            ps_bc = psS.tile([128, 1024], F32, tag="pss")
                            for half in range(2):
                                nc.tensor.matmul(
                                    ps_bc[0:64, 512 * half:512 * half + 512],
                                    ones_row_r[:, 0:64],
                                    recd[:, 512 * half:512 * half + 512],
                                    start=True, stop=True)
                            avn = avp.tile([64, 1024], FP8, tag="avn")
                            nc.vector.tensor_mul(avn[:], avs[0:64, :],
                                                 ps_bc[0:64, :])
                            g0 = 4 * b + 2 * qh
                            a2aih = a2a_pairs[h][0]
                            nc.sync.dma_start(a2aih[g0, :, :], avn[:, 0:TPC])
                            nc.sync.dma_start(a2aih[g0 + 1, :, :],
                                              avn[:, TPC:2 * TPC])
                        pending.append(epilogue)

                if b == B - 1:      # this head-half is complete
                    while pending:
                        pending.pop()()
                    if h == 0:      # launch early; h=1's goes after the
                        nc.gpsimd.collective_compute(   # pool scope closes
                            "AllToAll", ALU.bypass, replica_groups=RG,
                            ins=[a2ai0[:].opt()], outs=[a2ao0[:].opt()])

        qkvp_cm.__exit__(None, None, None)   # free Qh/Kh/Vt

        # ==== Phases 4-6 in one pool scope: everything here is emitted
        # before the second AllToAll so pool-open barriers don't serialize
        # on it; Wo's first half + weight prefetches overlap the collective.
        with tc.tile_pool(name="avtp", bufs=1) as avtp, \
             tc.tile_pool(name="w1p", bufs=2) as w1p, \
             tc.tile_pool(name="w2p", bufs=2) as w2p, \
             tc.tile_pool(name="tmp2", bufs=2) as tmp2p, \
             tc.tile_pool(name="sgp", bufs=2) as sgp, \
             tc.tile_pool(name="outp", bufs=2) as outp, \
             tc.tile_pool(name="psA2", bufs=8, space="PSUM") as psA2:
            psB = psA2
            w1_sb = []
            w2_sb = []
            for g in range(4):   # prefetch during the A2A / Wo phase
                w1t = w1p.tile([128, PO, 2048], FP8, tag="w1")
                nc.sync.dma_start(
                    w1t[:, :, 0:1024], wf1_in.ap()[:, 1024 * g:1024 * (g + 1)]
                    .rearrange("(po p) n -> p po n", p=128))
                nc.sync.dma_start(
                    w1t[:, :, 1024:2048],
                    wf1_in.ap()[:, DFF + 1024 * g:DFF + 1024 * (g + 1)]
                    .rearrange("(po p) n -> p po n", p=128))
                w1_sb.append(w1t)
            for g in range(4):
                w2t = w2p.tile([128, M1, 512], FP8, tag="w2")
                nc.sync.dma_start(
                    w2t[:], wf2_in.ap()[:, 512 * g:512 * (g + 1)]
                    .rearrange("(ko p) n -> p ko n", p=128))
                w2_sb.append(w2t)

            AVt0 = avtp.tile([128, 4, TPC], FP8, tag="avt0")
            nc.scalar.dma_start(
                AVt0[:],
                a2ao0[:].rearrange("(po s2) p t -> (s2 p) po t", s2=2))
            wo_ps = []
            for m in range(PO):   # h0 half: overlaps the second AllToAll
                ps = psA2.tile([128, TPC], F32, tag="ps")
                chi = slice(128 * m, 128 * m + 128)
                clo = slice(D + 128 * m, D + 128 * m + 128)
                for k in range(2):
                    ksl = slice(2 * k, 2 * k + 2)
                    nc.tensor.matmul(ps[:], wo_sb[:, ksl, chi],
                                     AVt0[:, ksl, :], start=(k == 0),
                                     stop=False, perf_mode=DR)
                    nc.tensor.matmul(ps[:], wo_sb[:, ksl, clo],
                                     AVt0[:, ksl, :], start=False,
                                     stop=False, perf_mode=DR)
                wo_ps.append(ps)

            nc.gpsimd.collective_compute(
                "AllToAll", ALU.bypass, replica_groups=RG,
                ins=[a2ai1[:].opt()], outs=[a2ao1[:].opt()])
            AVt1 = avtp.tile([128, 4, TPC], FP8, tag="avt1")
            nc.scalar.dma_start(
                AVt1[:],
                a2ao1[:].rearrange("(po s2) p t -> (s2 p) po t", s2=2))

            for m in range(PO):   # h1 half + epilogue
                ps = wo_ps[m]
                chi = slice(128 * m, 128 * m + 128)
                clo = slice(D + 128 * m, D + 128 * m + 128)
                for k in range(2):
                    ksl = slice(2 * k, 2 * k + 2)
                    nc.tensor.matmul(ps[:], wo_sb[:, 4 + 2 * k:6 + 2 * k, chi],
                                     AVt1[:, ksl, :], start=False,
                                     stop=False, perf_mode=DR)
                    nc.tensor.matmul(ps[:], wo_sb[:, 4 + 2 * k:6 + 2 * k, clo],
                                     AVt1[:, ksl, :], start=False,
                                     stop=False, perf_mode=DR)
                nc.tensor.matmul(ps[:], bo_r[:, 128 * m:128 * m + 128],
                                 ones_tok[:], start=False, stop=True)
                nc.vector.scalar_tensor_tensor(
                    out=X2[:, m, :].bitcast(F32), in0=ps[:],
                    scalar=1.0 / 16.0, in1=xl[:, m, :],
                    op0=ALU.mult, op1=ALU.add)

            ps_s2 = psB.tile([128, TPC], F32, tag="ps")
            for po in range(PO):
                nc.tensor.matmul(ps_s2[0:1, :], ones_col_r[:],
                                 X2[:, po, :],
                                 start=(po == 0), stop=(po == PO - 1))
            ps_q2 = psB.tile([128, TPC], F32, tag="ps")
            for po in range(PO):
                sq = tmp2p.tile([128, TPC], F32R, tag="sq2")
                nc.scalar.activation(sq[:].bitcast(F32),
                                     X2[:, po, :].bitcast(F32), AF.Square)
                nc.tensor.matmul(ps_q2[0:1, :], ones_col_r[:], sq[:],
                                 start=(po == 0), stop=(po == PO - 1))
            nmu2 = rows.tile([1, TPC], F32R, tag="nmu2")
            nc.scalar.activation(nmu2[:].bitcast(F32), ps_s2[0:1, :],
                                 AF.Copy, scale=-1.0 / D)
            ex2b = rows.tile([1, TPC], F32, tag="ex2b")
            nc.scalar.activation(ex2b[:], ps_q2[0:1, :], AF.Copy,
                                 scale=1.0 / D)
            mu2b = rows.tile([1, TPC], F32, tag="mu2b")
            nc.vector.tensor_mul(mu2b[:], nmu2[:].bitcast(F32),
                                 nmu2[:].bitcast(F32))
            varb = rows.tile([1, TPC], F32, tag="varb")
            nc.vector.scalar_tensor_tensor(
                out=varb[:], in0=ex2b[:], scalar=EPS, in1=mu2b[:],
                op0=ALU.add, op1=ALU.subtract)
            recb = rows.tile([1, TPC], F32, tag="recb")
            nc.vector.reciprocal(recb[:], varb[:])
            inv2 = rows.tile([1, TPC], F32R, tag="inv2")
            nc.scalar.activation(inv2[:].bitcast(F32), recb[:], AF.Sqrt)
            ps_mu = psB.tile([128, TPC], F32, tag="ps")
            nc.tensor.matmul(ps_mu[:], ones_row_r[:], nmu2[:],
                             start=True, stop=True)
            ps_iv = psB.tile([128, TPC], F32, tag="ps")
            nc.tensor.matmul(ps_iv[:], ones_row_r[:], inv2[:],
                             start=True, stop=True)
            for po in range(PO):
                t0 = tmp2p.tile([128, TPC], F32, tag="t0")
                nc.vector.tensor_add(t0[:], X2[:, po, :].bitcast(F32),
                                     ps_mu[:])
                nc.vector.tensor_mul(xh2[:, po, :], t0[:], ps_iv[:])

            for m in range(M1):
                ps = psB.tile([128, TPC], F32, tag="ps")
                w1t = w1_sb[m // 8]
                chi = slice(128 * (m % 8), 128 * (m % 8) + 128)
                clo = slice(1024 + 128 * (m % 8), 1024 + 128 * (m % 8) + 128)
                for k in range(PO // 2):
                    ksl = slice(2 * k, 2 * k + 2)
                    nc.tensor.matmul(ps[:], w1t[:, ksl, chi],
                                     xh2[:, ksl, :], start=(k == 0),
                                     stop=False, perf_mode=DR)
                    nc.tensor.matmul(ps[:], w1t[:, ksl, clo],
                                     xh2[:, ksl, :], start=False,
                                     stop=(k == PO // 2 - 1), perf_mode=DR)
                sg = sgp.tile([128, TPC], BF16, tag="sg")
                nc.scalar.activation(sg[:], ps[:], AF.Sigmoid,
                                     scale=SIGC / 16.0,
                                     bias=bf1s[:, m:m + 1])
                nc.vector.scalar_tensor_tensor(
                    out=Amat[:, m, :], in0=ps[:],
                    scalar=bf1[:, m:m + 1], in1=sg[:],
                    op0=ALU.add, op1=ALU.mult)
            out_view = out_t.ap().rearrange("(po p) t -> p po t", p=128)
            for m in range(PO):
                ps = psB.tile([128, TPC], F32, tag="ps")
                w2t = w2_sb[m // 2]
                chi = slice(128 * (m % 2), 128 * (m % 2) + 128)
                clo = slice(256 + 128 * (m % 2), 256 + 128 * (m % 2) + 128)
                for k in range(M1 // 2):
                    ksl = slice(2 * k, 2 * k + 2)
                    nc.tensor.matmul(ps[:], w2t[:, ksl, chi],
                                     Amat[:, ksl, :], start=(k == 0),
                                     stop=False, perf_mode=DR)
                    nc.tensor.matmul(ps[:], w2t[:, ksl, clo],
                                     Amat[:, ksl, :], start=False,
                                     stop=False, perf_mode=DR)
                nc.tensor.matmul(ps[:], bf2_r[:, 128 * m:128 * m + 128],
                                 ones_tok[:], start=False, stop=True)
                om = outp.tile([128, TPC], F32, tag="om")
                nc.vector.scalar_tensor_tensor(
                    out=om[:], in0=ps[:], scalar=1.0 / 256.0,
                    in1=X2[:, m, :].bitcast(F32),
                    op0=ALU.mult, op1=ALU.add)
                nc.sync.dma_start(out_view[:, m, :], om[:])
        xlp_cm.__exit__(None, None, None)    # free xl/wo_sb

    nc.compile()
    return nc


def _get_nc():
    key = ("nc", GELU_NATIVE)
    if key not in _CACHE:
        _CACHE[key] = _build()
    return _CACHE[key]


def _make_in_maps(inputs):
    x = np.asarray(inputs["x"], np.float32).reshape(BT, D)
    ln1w = np.asarray(inputs["ln1_w"], np.float32)
    ln1b = np.asarray(inputs["ln1_b"], np.float32)
    ln2w = np.asarray(inputs["ln2_w"], np.float32)
    ln2b = np.asarray(inputs["ln2_b"], np.float32)
    W_qkv0 = np.asarray(inputs["W_qkv"], np.float32)
    W_qkv = W_qkv0 * ln1w[:, None]
    b_qkv = np.asarray(inputs["b_qkv"], np.float32) + ln1b @ W_qkv0
    W_o = np.asarray(inputs["W_o"], np.float32)
    b_o = np.asarray(inputs["b_o"], np.float32)
    W_ff10 = np.asarray(inputs["W_ff1"], np.float32)
    W_ff1 = W_ff10 * ln2w[:, None]
    b_ff1 = np.asarray(inputs["b_ff1"], np.float32) + ln2b @ W_ff10
    W_ff2 = np.asarray(inputs["W_ff2"], np.float32)
    b_ff2 = np.asarray(inputs["b_ff2"], np.float32)

    def pcol(v):  # [D'] -> [128, D'/128] per-partition column layout
        return np.ascontiguousarray(v.reshape(-1, 128).T)

    xT = np.ascontiguousarray(x.T)                      # [D, BT] f32

    def hilo16(w):   # 16*w as fp8 hi + fp8 residual, concatenated wide
        base = (16.0 * w).astype(np.float32)
        hi = base.astype(NPFP8)
        lo = (base - hi.astype(np.float32)).astype(NPFP8)
        return np.ascontiguousarray(np.concatenate([hi, lo], axis=1))

    def hilo16_blocked(w, blk):   # [hi0|lo0|hi1|lo1|...] per blk columns
        base = (16.0 * w).astype(np.float32)
        hi = base.astype(NPFP8)
        lo = (base - hi.astype(np.float32)).astype(NPFP8)
        parts = []
        for g in range(w.shape[1] // blk):
            parts.append(hi[:, blk * g:blk * (g + 1)])
            parts.append(lo[:, blk * g:blk * (g + 1)])
        return np.ascontiguousarray(np.concatenate(parts, axis=1))

    common = {
        "xb": xT.astype(NPFP8),
        "wo": hilo16(W_o[WO_PERM]),
        "bo": np.ascontiguousarray(16.0 * b_o).reshape(1, D).astype(NPBF16),
        "wf1": hilo16(W_ff1),
        "bf1": pcol(16.0 * b_ff1),
        "bf1s": pcol(SIGC * b_ff1).astype(np.float32),
        "wf2": hilo16_blocked(W_ff2, 256),
        "bf2": np.ascontiguousarray(
            256.0 * b_ff2).reshape(1, D).astype(NPBF16),
    }
    in_maps = []
    for r in range(NC_N):
        hc = 128 * r          # first column of this core's Q/K/V head block
        m = dict(common)
        m["xs"] = np.ascontiguousarray(
            xT[:, TPC * r:TPC * (r + 1)]).astype(NPBF16)
        m["xf"] = np.ascontiguousarray(xT[:, TPC * r:TPC * (r + 1)])
        wq = W_qkv[:, hc:hc + 128]
        wk = W_qkv[:, D + hc:D + hc + 128]
        wv = W_qkv[:, 2 * D + hc:2 * D + hc + 128]
        m["wqkv"] = hilo16(np.concatenate([wq, wk, wv], axis=1))
        wdq = (m["wqkv"][:, 0:384].astype(np.float32)
               + m["wqkv"][:, 384:768].astype(np.float32))   # = 16*W'
        m["sqkv"] = np.ascontiguousarray(
            wdq[:, 0:256].sum(0).reshape(1, 256)).astype(np.float32)
        m["svb"] = np.ascontiguousarray(np.stack(
            [wdq[:, 256:384].sum(0),
             16.0 * b_qkv[2 * D + hc:2 * D + hc + 128]])).astype(NPBF16)
        m["bqkr"] = np.ascontiguousarray(16.0 * np.concatenate(
            [b_qkv[hc:hc + 128], b_qkv[D + hc:D + hc + 128]])
            .reshape(1, 256)).astype(np.float32)
        in_maps.append(m)
    return in_maps


def _run_sim(nc, in_maps):
    """Instruction-level simulator fallback executor (same program)."""
    from concourse.bass_interp import MultiCoreSim
    sim = MultiCoreSim(nc, num_cores=NC_N, require_finite=False)
    for i in range(NC_N):
        for k, v in in_maps[i].items():
            sim.cores[i].tensor(k)[:] = np.asarray(v)
    sim.simulate(check_with_hw=False)
    return [np.array(sim.cores[i].tensor("outt")) for i in range(NC_N)]


def _run(inputs, trace=False, trace_cores=None):
    nc = _get_nc()
    in_maps = _make_in_maps(inputs)
    res = None
    try:
        res = bass_utils.run_bass_kernel_spmd(
            nc, in_maps, core_ids=list(range(NC_N)), trace=trace,
            trace_cores=trace_cores)
        outs = [res.results[r]["outt"] for r in range(NC_N)]
    except Exception:
        outs = _run_sim(nc, in_maps)
    full = np.concatenate([np.asarray(o, np.float32).T for o in outs], axis=0)
    return full.reshape(B, T, D).astype(np.float32), res


def kernel(**inputs):
    out, _ = _run(inputs, trace=False)
    return out
